# revision 8
# baseline (speedup 1.0000x reference)
"""COIL-style sparse-attention scoring kernel for Trainium2 (8 NeuronCores).

Reference computation:
    scores[q,i,d,j] = <query_tok_embs[q,i], doc_tok_embs[d,j]>         (K=32)
    masked = where(query_ids[q,i]==doc_ids[d,j], scores, 0)
    tok    = masked.max(axis=j)
    tok_scores[q,d] = sum_i w[q,i] * tok[q,i,d]    (w drops CLS + SEP)
    out = tok_scores + query_cls_emb @ doc_cls_emb.T

Data-parallel over the 64 queries (8 per core, 2 row-blocks of 128 rows =
4 queries x 32 tokens); doc side replicated.

Device algorithm (v2 -- sum-decode, no big VectorE reduce):

  * One fp16 matmul per 512-col PSUM bank computes
      aug = score + 128*(#matching base-6 digit of the token ids) - 640
    The -640 full-match offset is folded in as a 63rd contraction row
    (query side -640, doc side 1).  A full 5-digit id match gives
    aug = score; any partial match stays <= score - 128 < 0.
  * relu(aug) is therefore exactly the where-masked score (or 0).  The
    per-(token,doc) max over doc positions j is replaced by a SUM of
    relu(aug) over j -- exact when each (token,doc) row has at most one
    matching j.  To make that hold, doc positions are permuted host-side
    (doc data only) into duplicate-rank groups: group r holds the r-th
    occurrence of each id within its doc, so ids are unique per doc within
    a group.  tok = elementwise max over the (few) group results.
  * The relu decode runs as fp32-PSUM -> fp8(e4m3)-SBUF tensor_scalar ops
    split across the Vector AND Scalar engines (the only PSUM-capable
    engines), roughly balancing their 0.96 / 1.2 GHz scan rates.
  * The j-sum becomes TensorE work: per phase-pair, a DoubleRow fp8 matmul
    with identity-pair weights accumulates r[.,2p] + r[.,2p+1] for all
    (row, doc) into a [128, 2*128] PSUM tile (192 j-phases -> 96 matmuls).
  * Group-max on VectorE (tiny: [128, 256] tiles), then the weighted sum
    over query tokens and the CLS scores are K=128 matmuls into one
    [8, 128] PSUM tile.
"""

import numpy as np
import ml_dtypes
from contextlib import ExitStack

import concourse.bass as bass
import concourse.bacc as bacc
import concourse.mybir as mybir
import concourse.tile as tile
from concourse.bass_utils import run_bass_kernel_spmd

F32 = mybir.dt.float32
F16 = mybir.dt.float16
BF16 = mybir.dt.bfloat16
FP8 = mybir.dt.float8e4

# problem shape (hardcoded per contract)
BQ, LQ, BD, LD, TOK_D, CLS_D = 64, 32, 128, 192, 32, 768
NCORES = 8
QPC = BQ // NCORES          # 8 queries per core
NBLK = 2                    # two row-blocks of 128 = 4 queries x 32 tokens
ROWS = 128
DIG = 6                     # digit base; 6^5 = 7776 > 5000 vocab
NDIG = 5
KD = NDIG * DIG             # 30 one-hot dims
KF = TOK_D + KD + 1         # 63 = emb + digit one-hots + offset row
C = 128.0                   # per-digit match bonus
OFF = NDIG * C              # 640 full-match offset
G0 = LD                     # rank-0 group is always padded to LD slots


def build_nc(gsizes, debug_taps=False):
    """gsizes: tuple of per-doc group slot counts, gsizes[0] == 192, rest even."""
    assert gsizes[0] == G0 and all(g % 2 == 0 for g in gsizes[1:])
    ngrp = len(gsizes)
    ndp = BD * sum(gsizes)              # doc-position columns per block
    reg_off = np.cumsum([0] + [BD * g for g in gsizes]).tolist()

    nc = bacc.Bacc(
        "TRN2",
        target_bir_lowering=False,
        debug=False,
        num_devices=NCORES,
    )

    qlhsT_d = nc.dram_tensor("qlhsT", [NBLK, KF, ROWS], F16, kind="ExternalInput")
    rhs_d = nc.dram_tensor("rhs", [KF, ndp], F16, kind="ExternalInput")
    sel_d = nc.dram_tensor("sel", [NBLK, ROWS, QPC], BF16, kind="ExternalInput")
    qclsT_d = nc.dram_tensor("qclsT", [CLS_D // 128, 128, QPC], BF16, kind="ExternalInput")
    dclsT_d = nc.dram_tensor("dclsT", [CLS_D // 128, 128, BD], BF16, kind="ExternalInput")
    idp_d = nc.dram_tensor("idp", [128, 2 * 128], FP8, kind="ExternalInput")
    out_d = nc.dram_tensor("out", [QPC, BD], F32, kind="ExternalOutput")
    if debug_taps:
        r0dbg_d = nc.dram_tensor("r0dbg", [128, NBLK * (gsizes[0] // 2) * BD * 2], FP8, kind="ExternalOutput")
        tokdbg_d = nc.dram_tensor("tokdbg", [128, NBLK * BD], BF16, kind="ExternalOutput")
        tdbg_d = nc.dram_tensor("tdbg", [128, ngrp * NBLK * BD], F32, kind="ExternalOutput")

    with tile.TileContext(nc) as tc, ExitStack() as ctx:
        const = ctx.enter_context(tc.tile_pool(name="const", bufs=1))
        psum = ctx.enter_context(tc.tile_pool(name="psum", bufs=2, space="PSUM"))
        tpsum = ctx.enter_context(tc.tile_pool(name="tpsum", bufs=1, space="PSUM"))
        work = ctx.enter_context(tc.tile_pool(name="work", bufs=1))

        # --- SBUF tiles ---
        # rhs doc features live twice: block 0 at partitions 0..62, block 1 at
        # 64..126 (enables PE row tiling -- two concurrent 512-col streams)
        rhs_t = const.tile([64 + KF, ndp], F16, tag="rhs")
        qlhsT_t = const.tile([64 + KF, NBLK * ROWS], F16, tag="qlhsT")
        sel_t = const.tile([ROWS, NBLK * QPC], BF16, tag="sel")
        qclsT_t = const.tile([128, 6 * QPC], BF16, tag="qclsT")
        dclsT_t = const.tile([128, 6 * BD], BF16, tag="dclsT")
        idp_t = const.tile([128, 2 * 128], FP8, tag="idp")
        # relu'd decode: r[p, (block, phase-pair, doc, pair-parity)] fp8
        r_ts = []
        for r in range(ngrp):
            r_t = const.tile(
                [128, NBLK * (gsizes[r] // 2) * BD * 2], FP8, tag=f"r{r}",
                name=f"r{r}",
            )
            r_ts.append(r_t)

        # --- input DMA: first group's columns first so PE starts ASAP ---
        for b in range(NBLK):
            nc.sync.dma_start(
                qlhsT_t[64 * b:64 * b + KF, b * ROWS:(b + 1) * ROWS], qlhsT_d[b]
            )
        dgrp = 3 * 512 // G0                      # 8 docs per 3-bank group
        cb = [0, 1536, 3072, 6144, 9216, 13824, 18432, 23040, ndp]
        engs = [nc.sync, nc.gpsimd, nc.scalar]
        for i in range(len(cb) - 1):
            c0, c1 = cb[i], cb[i + 1]
            if i == 0:
                nc.sync.dma_start(rhs_t[0:KF, c0:c1], rhs_d[:, c0:c1])
                nc.gpsimd.dma_start(rhs_t[64:64 + KF, c0:c1], rhs_d[:, c0:c1])
            else:
                e = engs[i % 3]
                e.dma_start(rhs_t[0:KF, c0:c1], rhs_d[:, c0:c1])
                engs[(i + 1) % 3].dma_start(rhs_t[64:64 + KF, c0:c1], rhs_d[:, c0:c1])
        nc.scalar.dma_start(idp_t[:], idp_d[:])
        for b in range(NBLK):
            nc.scalar.dma_start(sel_t[:, b * QPC:(b + 1) * QPC], sel_d[b])
        for k in range(6):
            nc.scalar.dma_start(qclsT_t[:, k * QPC:(k + 1) * QPC], qclsT_d[k])
            nc.gpsimd.dma_start(dclsT_t[:, k * BD:(k + 1) * BD], dclsT_d[k])

        # T tile also hosts the [8, 128] output accumulator as slot `ngrp`
        t_tile = tpsum.tile([128, ngrp + 1, NBLK, BD], F32, tag="T")
        out_ps = t_tile[0:QPC, ngrp, 0, :]

        # --- region 0: aug matmuls + relu decode, 3-bank groups of 8 docs ---
        ng0 = BD // dgrp                          # 16 groups
        ralt = 0
        for g in range(ng0):
            for b in range(NBLK):
                ps = psum.tile([128, 3, 512], F32, tag="aug")
                lhs = qlhsT_t[64 * b:64 * b + KF, b * ROWS:(b + 1) * ROWS]
                for k in range(3):
                    c0 = g * 1536 + k * 512
                    nc.tensor.matmul(
                        ps[:, k, :],
                        lhs,
                        rhs_t[64 * b:64 * b + KF, c0:c0 + 512],
                        start=True, stop=True,
                    )
                src = ps[:].rearrange("p a t -> p (a t)").rearrange(
                    "p (dl pp par) -> p dl pp par", dl=dgrp, par=2
                )
                dst = r_ts[0][:].rearrange(
                    "p (bb pp d par) -> p bb d pp par", bb=NBLK, d=BD, par=2
                )[:, b, g * dgrp:(g + 1) * dgrp, :, :]
                if ralt % 2 == 0:
                    nc.scalar.activation(dst, src, mybir.ActivationFunctionType.Relu)
                else:
                    nc.vector.tensor_scalar_max(dst, src, 0.0)
                ralt += 1

        # --- regions 1+: small dup-rank groups, one aug tile per (region, block) ---
        for r in range(1, ngrp):
            gr = gsizes[r]
            for b in range(NBLK):
                ps = psum.tile([128, 3, 512], F32, tag="aug")
                pr = ps[:].rearrange("p a t -> p (a t)")[:, 0:BD * gr]
                lhs = qlhsT_t[64 * b:64 * b + KF, b * ROWS:(b + 1) * ROWS]
                for c0 in range(0, BD * gr, 512):
                    cw = min(512, BD * gr - c0)
                    nc.tensor.matmul(
                        pr[:, c0:c0 + cw],
                        lhs,
                        rhs_t[64 * b:64 * b + KF, reg_off[r] + c0:reg_off[r] + c0 + cw],
                        start=True, stop=True,
                    )
                src = pr.rearrange("p (d pp par) -> p d pp par", d=BD, par=2)
                dst = r_ts[r][:].rearrange(
                    "p (bb pp d par) -> p bb d pp par", bb=NBLK, d=BD, par=2
                )[:, b, :, :, :]
                if r % 2 == 1:
                    nc.scalar.activation(dst, src, mybir.ActivationFunctionType.Relu)
                else:
                    nc.vector.tensor_scalar_max(dst, src, 0.0)

        # --- T matmuls: DoubleRow fp8, identity-pair weights, accumulate
        #     phase pairs into T[p, (block, doc)] ---
        idp_ap = idp_t[:].rearrange("p (o m) -> p o m", o=2)
        for r in range(ngrp):
            npp = gsizes[r] // 2
            for pp in range(npp):
                rhs_ap = r_ts[r][:].rearrange(
                    "p (bb pp d par) -> p pp par bb d", bb=NBLK, d=BD, par=2
                )[:, pp, :, :, :]
                nc.tensor.matmul(
                    t_tile[:, r, :, :], idp_ap, rhs_ap,
                    start=(pp == 0), stop=(pp == npp - 1),
                    perf_mode=mybir.MatmulPerfMode.DoubleRow,
                )

        # CLS matmuls: must start the out_ps accumulation group AFTER every
        # other start= in its PSUM bank (start clears has_written bank-wide)
        for k in range(6):
            nc.tensor.matmul(
                out_ps[:],
                qclsT_t[:, k * QPC:(k + 1) * QPC],
                dclsT_t[:, k * BD:(k + 1) * BD],
                start=(k == 0),
                stop=False,
            )

        # --- group max (exact reproduction of the reference per-row max) ---
        tok_t = work.tile([128, NBLK * BD], BF16, tag="tok")
        if ngrp == 1:
            nc.vector.tensor_copy(tok_t[:], t_tile[:, 0, :, :])
        else:
            acc = work.tile([128, NBLK * BD], BF16, tag="tmax_acc")
            nc.scalar.copy(acc[:], t_tile[:, ngrp - 1, :, :])
            for r in range(ngrp - 2, 0, -1):
                nxt = tok_t if r == 1 else work.tile(
                    [128, NBLK * BD], BF16, tag=f"tmax{r}"
                )
                nc.vector.tensor_tensor(
                    nxt[:], t_tile[:, r, :, :], acc[:], op=mybir.AluOpType.max
                )
                acc = nxt
            nc.vector.tensor_tensor(
                tok_t[:], t_tile[:, 0, :, :], acc[:], op=mybir.AluOpType.max
            )

        # --- weighted token sum into out_ps (continues CLS accumulation) ---
        for b in range(NBLK):
            nc.tensor.matmul(
                out_ps[:],
                sel_t[:, b * QPC:(b + 1) * QPC],
                tok_t[:, b * BD:(b + 1) * BD],
                start=False,
                stop=(b == NBLK - 1),
            )

        outsb = work.tile([QPC, BD], F32, tag="outsb")
        nc.scalar.copy(outsb[:], out_ps[:])
        nc.sync.dma_start(out_d[:], outsb[:])
        if debug_taps:
            nc.sync.dma_start(r0dbg_d[:], r_ts[0][:])
            nc.sync.dma_start(tokdbg_d[:], tok_t[:])
            tsb = work.tile([128, ngrp * NBLK * BD], F32, tag="tsb")
            for r in range(ngrp):
                nc.vector.tensor_copy(
                    tsb[:, r * NBLK * BD:(r + 1) * NBLK * BD], t_tile[:, r, :, :])
            nc.sync.dma_start(tdbg_d[:], tsb[:])

    nc.compile()
    return nc


_NC_CACHE = {}


def _get_nc(gsizes, debug_taps=False):
    key = (gsizes, debug_taps)
    if key not in _NC_CACHE:
        _NC_CACHE[key] = build_nc(gsizes, debug_taps)
    return _NC_CACHE[key]


def _digit_onehot(ids, scale):
    ids = ids.astype(np.int64)
    oh = np.zeros(ids.shape + (KD,), np.float32)
    flat = oh.reshape(-1, KD)
    fid = ids.reshape(-1)
    idx = np.arange(fid.size)
    for t in range(NDIG):
        flat[idx, t * DIG + (fid // (DIG ** t)) % DIG] = scale
    return oh


def _doc_groups(did):
    """Duplicate-rank grouping of doc positions (doc-side data only).

    Returns (gsizes, pos): gsizes[r] = per-doc slots for rank r (rank 0
    padded to LD, others rounded up to even); pos[r] = [BD, gsizes[r]]
    int array of source positions, -1 for padding."""
    ranks = np.zeros_like(did, dtype=np.int64)
    for d in range(BD):
        seen = {}
        for j in range(LD):
            v = int(did[d, j])
            r = seen.get(v, 0)
            seen[v] = r + 1
            ranks[d, j] = r
    nrank = int(ranks.max()) + 1
    gsizes = []
    pos = []
    for r in range(nrank):
        cnt = (ranks == r).sum(axis=1)
        gr = G0 if r == 0 else max(2, int(np.ceil(cnt.max() / 2)) * 2)
        p = np.full((BD, gr), -1, np.int64)
        for d in range(BD):
            js = np.nonzero(ranks[d] == r)[0]
            p[d, :len(js)] = js
        gsizes.append(gr)
        pos.append(p)
    return tuple(gsizes), pos


def make_in_maps(qte, dte, qce, dce, qid, did, qam):
    # SEP mask + CLS drop -> per-token weights
    sep = qam.sum(1) - 1
    qm = qam.astype(np.float32).copy()
    qm[np.arange(BQ), sep] = 0.0
    w = qm.copy()
    w[:, 0] = 0.0

    gsizes, pos = _doc_groups(did)
    ndp = BD * sum(gsizes)

    # doc-side feature matrix [KF, ndp], grouped column order, d-major per region
    doh = _digit_onehot(did, 1.0)                  # [BD, LD, KD]
    dfeat = np.concatenate(
        [dte.transpose(2, 0, 1).reshape(TOK_D, BD * LD),
         doh.transpose(2, 0, 1).reshape(KD, BD * LD),
         np.ones((1, BD * LD), np.float32)],
        axis=0,
    )                                              # [KF, BD*LD]
    rhs = np.zeros((KF, ndp), np.float16)
    off = 0
    for r, gr in enumerate(gsizes):
        idx = pos[r]                               # [BD, gr], -1 pad
        src = np.where(idx >= 0, np.arange(BD)[:, None] * LD + np.maximum(idx, 0), 0)
        block = dfeat[:, src.reshape(-1)].astype(np.float16)
        block[:, (idx < 0).reshape(-1)] = 0
        rhs[:, off:off + BD * gr] = block
        off += BD * gr

    qoh = _digit_onehot(qid, C)                    # [BQ, LQ, KD]
    dclsT = np.ascontiguousarray(
        dce.T.reshape(CLS_D // 128, 128, BD)).astype(ml_dtypes.bfloat16)
    idp = np.zeros((128, 2 * 128), dtype=ml_dtypes.float8_e4m3)
    for p in range(128):
        idp[p, p] = 1.0
        idp[p, 128 + p] = 1.0

    in_maps = []
    for c in range(NCORES):
        qs = slice(c * QPC, (c + 1) * QPC)
        qte_c, qoh_c, w_c = qte[qs], qoh[qs], w[qs]

        qlhsT = np.zeros((NBLK, KF, ROWS), np.float16)
        for b in range(NBLK):
            blk = qte_c[b * 4:(b + 1) * 4].reshape(ROWS, TOK_D)
            qlhsT[b, 0:TOK_D] = blk.astype(np.float16).T
            qlhsT[b, TOK_D:TOK_D + KD] = (
                qoh_c[b * 4:(b + 1) * 4].reshape(ROWS, KD).T.astype(np.float16)
            )
            qlhsT[b, KF - 1] = -OFF

        sel = np.zeros((NBLK, ROWS, QPC), np.float32)
        for b in range(NBLK):
            for qq in range(4):
                ql = b * 4 + qq
                sel[b, qq * 32:(qq + 1) * 32, ql] = w_c[ql]

        qclsT = np.ascontiguousarray(
            qce[qs].T.reshape(CLS_D // 128, 128, QPC)).astype(ml_dtypes.bfloat16)

        in_maps.append(
            {
                "qlhsT": qlhsT,
                "rhs": np.ascontiguousarray(rhs),
                "sel": sel.astype(ml_dtypes.bfloat16),
                "qclsT": qclsT,
                "dclsT": dclsT,
                "idp": idp,
            }
        )
    return gsizes, in_maps


def run(gsizes, in_maps, trace=False, debug_taps=False, **kwargs):
    nc = _get_nc(gsizes, debug_taps)
    return run_bass_kernel_spmd(
        nc, in_maps, core_ids=list(range(NCORES)), trace=trace, **kwargs
    )


def kernel(
    query_tok_embs,
    doc_tok_embs,
    query_cls_emb,
    doc_cls_emb,
    query_input_ids,
    doc_input_ids,
    query_attention_mask,
):
    qte = np.ascontiguousarray(np.asarray(query_tok_embs, np.float32))
    dte = np.ascontiguousarray(np.asarray(doc_tok_embs, np.float32))
    qce = np.ascontiguousarray(np.asarray(query_cls_emb, np.float32))
    dce = np.ascontiguousarray(np.asarray(doc_cls_emb, np.float32))
    qid = np.asarray(query_input_ids).astype(np.int64)
    did = np.asarray(doc_input_ids).astype(np.int64)
    qam = np.asarray(query_attention_mask).astype(np.int64)

    gsizes, in_maps = make_in_maps(qte, dte, qce, dce, qid, did, qam)
    res = run(gsizes, in_maps)
    out = np.concatenate([r["out"] for r in res.results], axis=0)
    return np.ascontiguousarray(out.astype(np.float32))


# revision 9
# speedup vs baseline: 1.0219x; 1.0219x over previous
"""COIL-style sparse-attention scoring kernel for Trainium2 (8 NeuronCores).

Reference computation:
    scores[q,i,d,j] = <query_tok_embs[q,i], doc_tok_embs[d,j]>         (K=32)
    masked = where(query_ids[q,i]==doc_ids[d,j], scores, 0)
    tok    = masked.max(axis=j)
    tok_scores[q,d] = sum_i w[q,i] * tok[q,i,d]    (w drops CLS + SEP)
    out = tok_scores + query_cls_emb @ doc_cls_emb.T

Data-parallel over the 64 queries (8 per core, 2 row-blocks of 128 rows =
4 queries x 32 tokens); doc side replicated.

Device algorithm (v2 -- sum-decode, no big VectorE reduce):

  * One fp16 matmul per 512-col PSUM bank computes
      aug = score + 128*(#matching base-6 digit of the token ids) - 640
    The -640 full-match offset is folded in as a 63rd contraction row
    (query side -640, doc side 1).  A full 5-digit id match gives
    aug = score; any partial match stays <= score - 128 < 0.
  * relu(aug) is therefore exactly the where-masked score (or 0).  The
    per-(token,doc) max over doc positions j is replaced by a SUM of
    relu(aug) over j -- exact when each (token,doc) row has at most one
    matching j.  To make that hold, doc positions are permuted host-side
    (doc data only) into duplicate-rank groups: group r holds the r-th
    occurrence of each id within its doc, so ids are unique per doc within
    a group.  tok = elementwise max over the (few) group results.
  * The relu decode runs as fp32-PSUM -> fp8(e4m3)-SBUF tensor_scalar ops
    split across the Vector AND Scalar engines (the only PSUM-capable
    engines), roughly balancing their 0.96 / 1.2 GHz scan rates.
  * The j-sum becomes TensorE work: per phase-pair, a DoubleRow fp8 matmul
    with identity-pair weights accumulates r[.,2p] + r[.,2p+1] for all
    (row, doc) into a [128, 2*128] PSUM tile (192 j-phases -> 96 matmuls).
  * Group-max on VectorE (tiny: [128, 256] tiles), then the weighted sum
    over query tokens and the CLS scores are K=128 matmuls into one
    [8, 128] PSUM tile.
"""

import numpy as np
import ml_dtypes
from contextlib import ExitStack

import concourse.bass as bass
import concourse.bacc as bacc
import concourse.mybir as mybir
import concourse.tile as tile
from concourse.bass_utils import run_bass_kernel_spmd

F32 = mybir.dt.float32
F16 = mybir.dt.float16
BF16 = mybir.dt.bfloat16
FP8 = mybir.dt.float8e4

# problem shape (hardcoded per contract)
BQ, LQ, BD, LD, TOK_D, CLS_D = 64, 32, 128, 192, 32, 768
NCORES = 8
QPC = BQ // NCORES          # 8 queries per core
NBLK = 2                    # two row-blocks of 128 = 4 queries x 32 tokens
ROWS = 128
DIG = 6                     # digit base; 6^5 = 7776 > 5000 vocab
NDIG = 5
KD = NDIG * DIG             # 30 one-hot dims
KF = TOK_D + KD + 1         # 63 = emb + digit one-hots + offset row
C = 128.0                   # per-digit match bonus
OFF = NDIG * C              # 640 full-match offset
G0 = LD                     # rank-0 group is always padded to LD slots


def build_nc(gsizes, debug_taps=False):
    """gsizes: tuple of per-doc group slot counts, gsizes[0] == 192, rest even."""
    assert gsizes[0] == G0 and all(g % 2 == 0 for g in gsizes[1:])
    ngrp = len(gsizes)
    ndp = BD * sum(gsizes)              # doc-position columns per block
    reg_off = np.cumsum([0] + [BD * g for g in gsizes]).tolist()

    nc = bacc.Bacc(
        "TRN2",
        target_bir_lowering=False,
        debug=False,
        num_devices=NCORES,
    )

    qlhsT_d = nc.dram_tensor("qlhsT", [NBLK, KF, ROWS], F16, kind="ExternalInput")
    rhs_d = nc.dram_tensor("rhs", [KF, ndp], F16, kind="ExternalInput")
    sel_d = nc.dram_tensor("sel", [NBLK, ROWS, QPC], BF16, kind="ExternalInput")
    qclsT_d = nc.dram_tensor("qclsT", [CLS_D // 128, 128, QPC], BF16, kind="ExternalInput")
    dclsT_d = nc.dram_tensor("dclsT", [CLS_D // 128, 128, BD], BF16, kind="ExternalInput")
    idp_d = nc.dram_tensor("idp", [128, 2 * 128], FP8, kind="ExternalInput")
    out_d = nc.dram_tensor("out", [QPC, BD], F32, kind="ExternalOutput")
    if debug_taps:
        r0dbg_d = nc.dram_tensor("r0dbg", [128, NBLK * (gsizes[0] // 2) * BD * 2], FP8, kind="ExternalOutput")
        tokdbg_d = nc.dram_tensor("tokdbg", [128, NBLK * BD], BF16, kind="ExternalOutput")
        tdbg_d = nc.dram_tensor("tdbg", [128, ngrp * NBLK * BD], F32, kind="ExternalOutput")

    with tile.TileContext(nc) as tc, ExitStack() as ctx:
        const = ctx.enter_context(tc.tile_pool(name="const", bufs=1))
        psum = ctx.enter_context(tc.tile_pool(name="psum", bufs=2, space="PSUM"))
        tpsum = ctx.enter_context(tc.tile_pool(name="tpsum", bufs=1, space="PSUM"))
        work = ctx.enter_context(tc.tile_pool(name="work", bufs=1))

        # --- SBUF tiles ---
        # rhs doc features live twice: block 0 at partitions 0..62, block 1 at
        # 64..126 (enables PE row tiling -- two concurrent 512-col streams)
        rhs_t = const.tile([64 + KF, ndp], F16, tag="rhs")
        qlhsT_t = const.tile([64 + KF, NBLK * ROWS], F16, tag="qlhsT")
        sel_t = const.tile([ROWS, NBLK * QPC], BF16, tag="sel")
        qclsT_t = const.tile([128, 6 * QPC], BF16, tag="qclsT")
        dclsT_t = const.tile([128, 6 * BD], BF16, tag="dclsT")
        idp_t = const.tile([128, 2 * 128], FP8, tag="idp")
        # relu'd decode, raw PSUM column order:
        #   r0[p, (block, group, doc-in-group, phase)]   (phase innermost)
        #   r1+[p, (block, doc, slot)]                   (slot innermost)
        r_ts = []
        for r in range(ngrp):
            r_t = const.tile(
                [128, NBLK * (gsizes[r] // 2) * BD * 2], FP8, tag=f"r{r}",
                name=f"r{r}",
            )
            r_ts.append(r_t)

        # --- input DMA: first group's columns first so PE starts ASAP ---
        for b in range(NBLK):
            nc.sync.dma_start(
                qlhsT_t[64 * b:64 * b + KF, b * ROWS:(b + 1) * ROWS], qlhsT_d[b]
            )
        dgrp = 3 * 512 // G0                      # 8 docs per 3-bank group
        cb = [0, 1536, 3072, 6144, 9216, 13824, 18432, 23040, ndp]
        engs = [nc.sync, nc.gpsimd, nc.scalar]
        for i in range(len(cb) - 1):
            c0, c1 = cb[i], cb[i + 1]
            if i == 0:
                nc.sync.dma_start(rhs_t[0:KF, c0:c1], rhs_d[:, c0:c1])
                nc.gpsimd.dma_start(rhs_t[64:64 + KF, c0:c1], rhs_d[:, c0:c1])
            else:
                e = engs[i % 3]
                e.dma_start(rhs_t[0:KF, c0:c1], rhs_d[:, c0:c1])
                engs[(i + 1) % 3].dma_start(rhs_t[64:64 + KF, c0:c1], rhs_d[:, c0:c1])
        nc.scalar.dma_start(idp_t[:], idp_d[:])
        for b in range(NBLK):
            nc.scalar.dma_start(sel_t[:, b * QPC:(b + 1) * QPC], sel_d[b])
        for k in range(6):
            nc.scalar.dma_start(qclsT_t[:, k * QPC:(k + 1) * QPC], qclsT_d[k])
            nc.gpsimd.dma_start(dclsT_t[:, k * BD:(k + 1) * BD], dclsT_d[k])

        # T tile also hosts the [8, 128] output accumulator as slot `ngrp`
        t_tile = tpsum.tile([128, ngrp + 1, NBLK, BD], F32, tag="T")
        out_ps = t_tile[0:QPC, ngrp, 0, :]

        # --- region 0: aug matmuls + relu decode, 3-bank groups of 8 docs ---
        ng0 = BD // dgrp                          # 16 groups
        ralt = 0
        for g in range(ng0):
            for b in range(NBLK):
                ps = psum.tile([128, 3, 512], F32, tag="aug")
                lhs = qlhsT_t[64 * b:64 * b + KF, b * ROWS:(b + 1) * ROWS]
                for k in range(3):
                    c0 = g * 1536 + k * 512
                    nc.tensor.matmul(
                        ps[:, k, :],
                        lhs,
                        rhs_t[64 * b:64 * b + KF, c0:c0 + 512],
                        start=True, stop=True,
                    )
                src = ps[:].rearrange("p a t -> p (a t)")
                nslab = dgrp * G0
                dst = r_ts[0][:, (b * ng0 + g) * nslab:(b * ng0 + g + 1) * nslab]
                if ralt % 2 == 0:
                    nc.scalar.activation(dst, src, mybir.ActivationFunctionType.Relu)
                else:
                    nc.vector.tensor_scalar_max(dst, src, 0.0)
                ralt += 1

        # --- regions 1+: small dup-rank groups, one aug tile per (region, block) ---
        for r in range(1, ngrp):
            gr = gsizes[r]
            for b in range(NBLK):
                ps = psum.tile([128, 3, 512], F32, tag="aug")
                pr = ps[:].rearrange("p a t -> p (a t)")[:, 0:BD * gr]
                lhs = qlhsT_t[64 * b:64 * b + KF, b * ROWS:(b + 1) * ROWS]
                for c0 in range(0, BD * gr, 512):
                    cw = min(512, BD * gr - c0)
                    nc.tensor.matmul(
                        pr[:, c0:c0 + cw],
                        lhs,
                        rhs_t[64 * b:64 * b + KF, reg_off[r] + c0:reg_off[r] + c0 + cw],
                        start=True, stop=True,
                    )
                src = pr
                dst = r_ts[r][:, b * BD * gr:(b + 1) * BD * gr]
                if r % 2 == 1:
                    nc.scalar.activation(dst, src, mybir.ActivationFunctionType.Relu)
                else:
                    nc.vector.tensor_scalar_max(dst, src, 0.0)

        # --- T matmuls: DoubleRow fp8, identity-pair weights, accumulate
        #     phase pairs into T[p, (block, doc)] ---
        idp_ap = idp_t[:].rearrange("p (o m) -> p o m", o=2)
        for r in range(ngrp):
            gr = gsizes[r]
            npp = gr // 2
            for pp in range(npp):
                # [p, par(2, str 1), (b,d)(256, str gr)] at offset 2*pp
                rhs_ap = r_ts[r][:].rearrange(
                    "p (bd par) -> p bd par", par=gr
                )[:, :, 2 * pp:2 * pp + 2].rearrange("p bd par -> p par bd")
                nc.tensor.matmul(
                    t_tile[:, r, :, :], idp_ap, rhs_ap,
                    start=(pp == 0), stop=(pp == npp - 1),
                    perf_mode=mybir.MatmulPerfMode.DoubleRow,
                )

        # CLS matmuls: must start the out_ps accumulation group AFTER every
        # other start= in its PSUM bank (start clears has_written bank-wide)
        for k in range(6):
            nc.tensor.matmul(
                out_ps[:],
                qclsT_t[:, k * QPC:(k + 1) * QPC],
                dclsT_t[:, k * BD:(k + 1) * BD],
                start=(k == 0),
                stop=False,
            )

        # --- group max (exact reproduction of the reference per-row max) ---
        tok_t = work.tile([128, NBLK * BD], BF16, tag="tok")
        if ngrp == 1:
            nc.vector.tensor_copy(tok_t[:], t_tile[:, 0, :, :])
        else:
            acc = work.tile([128, NBLK * BD], BF16, tag="tmax_acc")
            nc.scalar.copy(acc[:], t_tile[:, ngrp - 1, :, :])
            for r in range(ngrp - 2, 0, -1):
                nxt = tok_t if r == 1 else work.tile(
                    [128, NBLK * BD], BF16, tag=f"tmax{r}"
                )
                nc.vector.tensor_tensor(
                    nxt[:], t_tile[:, r, :, :], acc[:], op=mybir.AluOpType.max
                )
                acc = nxt
            nc.vector.tensor_tensor(
                tok_t[:], t_tile[:, 0, :, :], acc[:], op=mybir.AluOpType.max
            )

        # --- weighted token sum into out_ps (continues CLS accumulation) ---
        for b in range(NBLK):
            nc.tensor.matmul(
                out_ps[:],
                sel_t[:, b * QPC:(b + 1) * QPC],
                tok_t[:, b * BD:(b + 1) * BD],
                start=False,
                stop=(b == NBLK - 1),
            )

        outsb = work.tile([QPC, BD], F32, tag="outsb")
        nc.scalar.copy(outsb[:], out_ps[:])
        nc.sync.dma_start(out_d[:], outsb[:])
        if debug_taps:
            nc.sync.dma_start(r0dbg_d[:], r_ts[0][:])
            nc.sync.dma_start(tokdbg_d[:], tok_t[:])
            tsb = work.tile([128, ngrp * NBLK * BD], F32, tag="tsb")
            for r in range(ngrp):
                nc.vector.tensor_copy(
                    tsb[:, r * NBLK * BD:(r + 1) * NBLK * BD], t_tile[:, r, :, :])
            nc.sync.dma_start(tdbg_d[:], tsb[:])

    nc.compile()
    return nc


_NC_CACHE = {}


def _get_nc(gsizes, debug_taps=False):
    key = (gsizes, debug_taps)
    if key not in _NC_CACHE:
        _NC_CACHE[key] = build_nc(gsizes, debug_taps)
    return _NC_CACHE[key]


def _digit_onehot(ids, scale):
    ids = ids.astype(np.int64)
    oh = np.zeros(ids.shape + (KD,), np.float32)
    flat = oh.reshape(-1, KD)
    fid = ids.reshape(-1)
    idx = np.arange(fid.size)
    for t in range(NDIG):
        flat[idx, t * DIG + (fid // (DIG ** t)) % DIG] = scale
    return oh


def _doc_groups(did):
    """Duplicate-rank grouping of doc positions (doc-side data only).

    Returns (gsizes, pos): gsizes[r] = per-doc slots for rank r (rank 0
    padded to LD, others rounded up to even); pos[r] = [BD, gsizes[r]]
    int array of source positions, -1 for padding."""
    ranks = np.zeros_like(did, dtype=np.int64)
    for d in range(BD):
        seen = {}
        for j in range(LD):
            v = int(did[d, j])
            r = seen.get(v, 0)
            seen[v] = r + 1
            ranks[d, j] = r
    nrank = int(ranks.max()) + 1
    gsizes = []
    pos = []
    for r in range(nrank):
        cnt = (ranks == r).sum(axis=1)
        gr = G0 if r == 0 else max(2, int(np.ceil(cnt.max() / 2)) * 2)
        p = np.full((BD, gr), -1, np.int64)
        for d in range(BD):
            js = np.nonzero(ranks[d] == r)[0]
            p[d, :len(js)] = js
        gsizes.append(gr)
        pos.append(p)
    return tuple(gsizes), pos


def make_in_maps(qte, dte, qce, dce, qid, did, qam):
    # SEP mask + CLS drop -> per-token weights
    sep = qam.sum(1) - 1
    qm = qam.astype(np.float32).copy()
    qm[np.arange(BQ), sep] = 0.0
    w = qm.copy()
    w[:, 0] = 0.0

    gsizes, pos = _doc_groups(did)
    ndp = BD * sum(gsizes)

    # doc-side feature matrix [KF, ndp], grouped column order, d-major per region
    doh = _digit_onehot(did, 1.0)                  # [BD, LD, KD]
    dfeat = np.concatenate(
        [dte.transpose(2, 0, 1).reshape(TOK_D, BD * LD),
         doh.transpose(2, 0, 1).reshape(KD, BD * LD),
         np.ones((1, BD * LD), np.float32)],
        axis=0,
    )                                              # [KF, BD*LD]
    rhs = np.zeros((KF, ndp), np.float16)
    off = 0
    for r, gr in enumerate(gsizes):
        idx = pos[r]                               # [BD, gr], -1 pad
        src = np.where(idx >= 0, np.arange(BD)[:, None] * LD + np.maximum(idx, 0), 0)
        block = dfeat[:, src.reshape(-1)].astype(np.float16)
        block[:, (idx < 0).reshape(-1)] = 0
        rhs[:, off:off + BD * gr] = block
        off += BD * gr

    qoh = _digit_onehot(qid, C)                    # [BQ, LQ, KD]
    dclsT = np.ascontiguousarray(
        dce.T.reshape(CLS_D // 128, 128, BD)).astype(ml_dtypes.bfloat16)
    idp = np.zeros((128, 2 * 128), dtype=ml_dtypes.float8_e4m3)
    for p in range(128):
        idp[p, p] = 1.0
        idp[p, 128 + p] = 1.0

    in_maps = []
    for c in range(NCORES):
        qs = slice(c * QPC, (c + 1) * QPC)
        qte_c, qoh_c, w_c = qte[qs], qoh[qs], w[qs]

        qlhsT = np.zeros((NBLK, KF, ROWS), np.float16)
        for b in range(NBLK):
            blk = qte_c[b * 4:(b + 1) * 4].reshape(ROWS, TOK_D)
            qlhsT[b, 0:TOK_D] = blk.astype(np.float16).T
            qlhsT[b, TOK_D:TOK_D + KD] = (
                qoh_c[b * 4:(b + 1) * 4].reshape(ROWS, KD).T.astype(np.float16)
            )
            qlhsT[b, KF - 1] = -OFF

        sel = np.zeros((NBLK, ROWS, QPC), np.float32)
        for b in range(NBLK):
            for qq in range(4):
                ql = b * 4 + qq
                sel[b, qq * 32:(qq + 1) * 32, ql] = w_c[ql]

        qclsT = np.ascontiguousarray(
            qce[qs].T.reshape(CLS_D // 128, 128, QPC)).astype(ml_dtypes.bfloat16)

        in_maps.append(
            {
                "qlhsT": qlhsT,
                "rhs": np.ascontiguousarray(rhs),
                "sel": sel.astype(ml_dtypes.bfloat16),
                "qclsT": qclsT,
                "dclsT": dclsT,
                "idp": idp,
            }
        )
    return gsizes, in_maps


def run(gsizes, in_maps, trace=False, debug_taps=False, **kwargs):
    nc = _get_nc(gsizes, debug_taps)
    return run_bass_kernel_spmd(
        nc, in_maps, core_ids=list(range(NCORES)), trace=trace, **kwargs
    )


def kernel(
    query_tok_embs,
    doc_tok_embs,
    query_cls_emb,
    doc_cls_emb,
    query_input_ids,
    doc_input_ids,
    query_attention_mask,
):
    qte = np.ascontiguousarray(np.asarray(query_tok_embs, np.float32))
    dte = np.ascontiguousarray(np.asarray(doc_tok_embs, np.float32))
    qce = np.ascontiguousarray(np.asarray(query_cls_emb, np.float32))
    dce = np.ascontiguousarray(np.asarray(doc_cls_emb, np.float32))
    qid = np.asarray(query_input_ids).astype(np.int64)
    did = np.asarray(doc_input_ids).astype(np.int64)
    qam = np.asarray(query_attention_mask).astype(np.int64)

    gsizes, in_maps = make_in_maps(qte, dte, qce, dce, qid, did, qam)
    res = run(gsizes, in_maps)
    out = np.concatenate([r["out"] for r in res.results], axis=0)
    return np.ascontiguousarray(out.astype(np.float32))


# revision 11
# speedup vs baseline: 1.2676x; 1.2405x over previous
"""COIL-style sparse-attention scoring kernel for Trainium2 (8 NeuronCores).

Reference computation:
    scores[q,i,d,j] = <query_tok_embs[q,i], doc_tok_embs[d,j]>         (K=32)
    masked = where(query_ids[q,i]==doc_ids[d,j], scores, 0)
    tok    = masked.max(axis=j)
    tok_scores[q,d] = sum_i w[q,i] * tok[q,i,d]    (w drops CLS + SEP)
    out = tok_scores + query_cls_emb @ doc_cls_emb.T

Data-parallel over the 64 queries (8 per core, 2 row-blocks of 128 rows =
4 queries x 32 tokens); doc side replicated.

Device algorithm (v3 -- fp8 DoubleRow + sum-decode):

  * The cartesian score+match matmul runs as fp8(e4m3) DoubleRow at 2
    cols/cycle with K=96 packed as 48 partition-pairs:
      pairs 0..31: query (q_hi[e], q_lo[e])  x  doc (d8[e], d8[e])
      pairs 32..46: query 128*onehot pairs   x  doc onehot pairs
      pair  47:    query (-160, -160)        x  doc (2, 2)
    giving  aug = score + 128*(#matching base-6 id digits) - 640  in PSUM.
    A full 5-digit id match makes aug = score; otherwise aug <= score-128.
  * relu(aug) == the where-masked score.  The per-(token,doc) max over doc
    positions j is replaced by a SUM of relu(aug) over j, exact because doc
    positions are permuted host-side (doc data only) into duplicate-rank
    groups: within a group no id repeats inside a doc, so each (token,doc)
    row has at most one match per group.  tok = max over the few groups.
  * relu decode: fp32-PSUM -> fp8-SBUF tensor_scalar/activation split
    across Vector AND Scalar engines (the only PSUM-readers), contiguous
    writes in raw PSUM order.
  * j-sum on TensorE: per phase-pair, one fp8 DoubleRow matmul with
    identity-pair weights accumulates r[.,2p]+r[.,2p+1] over all (row,doc)
    into a [128, 2*128] PSUM tile; the strided rhs AP does the reorder.
  * Group-max on VectorE ([128, 256] tiles), then CLS (bf16 K=768) and the
    weighted token sum (K=128) accumulate into one [8, 128] PSUM tile.
"""

import numpy as np
import ml_dtypes
from contextlib import ExitStack

import concourse.bass as bass
import concourse.bacc as bacc
import concourse.mybir as mybir
import concourse.tile as tile
from concourse.bass_utils import run_bass_kernel_spmd

F32 = mybir.dt.float32
F16 = mybir.dt.float16
BF16 = mybir.dt.bfloat16
FP8 = mybir.dt.float8e4
E4 = ml_dtypes.float8_e4m3

# problem shape (hardcoded per contract)
BQ, LQ, BD, LD, TOK_D, CLS_D = 64, 32, 128, 192, 32, 768
NCORES = 8
QPC = BQ // NCORES          # 8 queries per core
NBLK = 2                    # two row-blocks of 128 = 4 queries x 32 tokens
ROWS = 128
DIG = 6                     # digit base; 6^5 = 7776 > 5000 vocab
NDIG = 5
KD = NDIG * DIG             # 30 one-hot dims
KP = TOK_D + KD // 2 + 1    # 48 partition-pairs (K=96)
C = 128.0                   # per-digit match bonus
OFF = NDIG * C              # 640 full-match offset
BIAS = -OFF / 4             # -160: fp8-exact (|x|<=240), x2 via the pair
                            # and x2 via the doc-side bias value of 2.0
G0 = LD                     # rank-0 group is always padded to LD slots


def _chunks(ndp):
    """rhs DMA chunk boundaries (cols); small leading chunk, 1536-aligned."""
    cb = [0, 1536, 4608]
    while cb[-1] + 4608 < ndp:
        cb.append(cb[-1] + 4608)
    cb.append(ndp)
    return cb


def build_nc(gsizes, debug_taps=False):
    """gsizes: tuple of per-doc group slot counts, gsizes[0] == 192, rest even."""
    assert gsizes[0] == G0 and all(g % 2 == 0 for g in gsizes[1:])
    ngrp = len(gsizes)
    ndp = BD * sum(gsizes)              # doc-position columns per block
    reg_off = np.cumsum([0] + [BD * g for g in gsizes]).tolist()
    cb = _chunks(ndp)

    nc = bacc.Bacc(
        "TRN2",
        target_bir_lowering=False,
        debug=False,
        num_devices=NCORES,
    )

    # qlhsT[b]: [KP, 2, ROWS] fp8 pair-major weights per block
    qlhsT_d = nc.dram_tensor("qlhsT", [NBLK, KP, 2 * ROWS], FP8, kind="ExternalInput")
    # rhs chunk-major: chunk c holds [KP, w_c * 2] fp8 (pair slot innermost)
    rhs_ds = [
        nc.dram_tensor(f"rhs{i}", [KP, (cb[i + 1] - cb[i]) * 2], FP8,
                       kind="ExternalInput")
        for i in range(len(cb) - 1)
    ]
    sel_d = nc.dram_tensor("sel", [NBLK, ROWS, QPC], BF16, kind="ExternalInput")
    qclsT_d = nc.dram_tensor("qclsT", [CLS_D // 128, 128, QPC], BF16, kind="ExternalInput")
    dclsT_d = nc.dram_tensor("dclsT", [CLS_D // 128, 128, BD], BF16, kind="ExternalInput")
    idp_d = nc.dram_tensor("idp", [128, 2 * 128], FP8, kind="ExternalInput")
    out_d = nc.dram_tensor("out", [QPC, BD], F32, kind="ExternalOutput")
    if debug_taps:
        r0dbg_d = nc.dram_tensor("r0dbg", [128, NBLK * BD * gsizes[0]], FP8, kind="ExternalOutput")
        tokdbg_d = nc.dram_tensor("tokdbg", [128, NBLK * BD], BF16, kind="ExternalOutput")
        tdbg_d = nc.dram_tensor("tdbg", [128, ngrp * NBLK * BD], F32, kind="ExternalOutput")

    with tile.TileContext(nc) as tc, ExitStack() as ctx:
        const = ctx.enter_context(tc.tile_pool(name="const", bufs=1))
        psum = ctx.enter_context(tc.tile_pool(name="psum", bufs=2, space="PSUM"))
        tpsum = ctx.enter_context(tc.tile_pool(name="tpsum", bufs=1, space="PSUM"))
        work = ctx.enter_context(tc.tile_pool(name="work", bufs=1))

        # --- SBUF tiles ---
        rhs_t = const.tile([KP, 2 * ndp], FP8, tag="rhs")       # [p, (col, pair)]
        qlhsT_t = const.tile([KP, NBLK * 2 * ROWS], FP8, tag="qlhsT")
        sel_t = const.tile([ROWS, NBLK * QPC], BF16, tag="sel")
        qclsT_t = const.tile([128, 6 * QPC], BF16, tag="qclsT")
        dclsT_t = const.tile([128, 6 * BD], BF16, tag="dclsT")
        idp_t = const.tile([128, 2 * 128], FP8, tag="idp")
        # relu'd decode, raw PSUM column order:
        #   r0[p, (block, group, doc-in-group, phase)]   (phase innermost)
        #   r1+[p, (block, doc, slot)]                   (slot innermost)
        r_ts = []
        for r in range(ngrp):
            r_t = const.tile(
                [128, NBLK * BD * gsizes[r]], FP8, tag=f"r{r}", name=f"r{r}",
            )
            r_ts.append(r_t)

        # --- input DMA: rhs on the two HWDGE queues (sync+scalar), first
        # chunk split across both; small tensors trickle on gpsimd SWDGE ---
        for b in range(NBLK):
            nc.sync.dma_start(
                qlhsT_t[:, b * 2 * ROWS:(b + 1) * 2 * ROWS], qlhsT_d[b]
            )
        half0 = cb[1] // 2
        nc.sync.dma_start(rhs_t[:, 0:2 * half0], rhs_ds[0][:, 0:2 * half0])
        nc.scalar.dma_start(rhs_t[:, 2 * half0:2 * cb[1]],
                            rhs_ds[0][:, 2 * half0:2 * cb[1]])
        engs = [nc.scalar, nc.sync]
        for i in range(1, len(cb) - 1):
            engs[i % 2].dma_start(rhs_t[:, 2 * cb[i]:2 * cb[i + 1]], rhs_ds[i][:])
        nc.gpsimd.dma_start(idp_t[:], idp_d[:])
        for b in range(NBLK):
            nc.gpsimd.dma_start(sel_t[:, b * QPC:(b + 1) * QPC], sel_d[b])
        for k in range(6):
            nc.gpsimd.dma_start(qclsT_t[:, k * QPC:(k + 1) * QPC], qclsT_d[k])
            nc.gpsimd.dma_start(dclsT_t[:, k * BD:(k + 1) * BD], dclsT_d[k])

        # T tile also hosts the [8, 128] output accumulator as slot `ngrp`
        t_tile = tpsum.tile([128, ngrp + 1, NBLK, BD], F32, tag="T")
        out_ps = t_tile[0:QPC, ngrp, 0, :]

        rhs_pairs = rhs_t[:].rearrange("p (n o) -> p o n", o=2)

        def aug_mm(ps_slice, b, c0, cw):
            nc.tensor.matmul(
                ps_slice,
                qlhsT_t[:, b * 2 * ROWS:(b + 1) * 2 * ROWS].rearrange(
                    "p (o m) -> p o m", o=2),
                rhs_pairs[:, :, c0:c0 + cw],
                start=True, stop=True,
                perf_mode=mybir.MatmulPerfMode.DoubleRow,
            )

        # --- region 0: aug matmuls + relu decode, 3-bank groups of 8 docs ---
        dgrp = 3 * 512 // G0                      # 8 docs per 3-bank group
        ng0 = BD // dgrp                          # 16 groups
        ralt = 0
        for g in range(ng0):
            for b in range(NBLK):
                ps = psum.tile([128, 3, 512], F32, tag="aug")
                for k in range(3):
                    aug_mm(ps[:, k, :], b, g * 1536 + k * 512, 512)
                src = ps[:].rearrange("p a t -> p (a t)")
                nslab = dgrp * G0
                dst = r_ts[0][:, (b * ng0 + g) * nslab:(b * ng0 + g + 1) * nslab]
                if ralt % 2 == 0:
                    nc.scalar.activation(dst, src, mybir.ActivationFunctionType.Relu)
                else:
                    nc.vector.tensor_scalar_max(dst, src, 0.0)
                ralt += 1

        # --- regions 1+: small dup-rank groups ---
        for r in range(1, ngrp):
            gr = gsizes[r]
            for b in range(NBLK):
                ps = psum.tile([128, 3, 512], F32, tag="aug")
                pr = ps[:].rearrange("p a t -> p (a t)")[:, 0:BD * gr]
                for c0 in range(0, BD * gr, 512):
                    cw = min(512, BD * gr - c0)
                    aug_mm(pr[:, c0:c0 + cw], b, reg_off[r] + c0, cw)
                dst = r_ts[r][:, b * BD * gr:(b + 1) * BD * gr]
                if r % 2 == 1:
                    nc.scalar.activation(dst, pr, mybir.ActivationFunctionType.Relu)
                else:
                    nc.vector.tensor_scalar_max(dst, pr, 0.0)

        # --- T matmuls: DoubleRow fp8, identity-pair weights, accumulate
        #     phase pairs into T[p, (block, doc)]; strided rhs AP reorders ---
        idp_ap = idp_t[:].rearrange("p (o m) -> p o m", o=2)
        for r in range(ngrp):
            gr = gsizes[r]
            npp = gr // 2
            for pp in range(npp):
                rhs_ap = r_ts[r][:].rearrange(
                    "p (bd par) -> p bd par", par=gr
                )[:, :, 2 * pp:2 * pp + 2].rearrange("p bd par -> p par bd")
                nc.tensor.matmul(
                    t_tile[:, r, :, :], idp_ap, rhs_ap,
                    start=(pp == 0), stop=(pp == npp - 1),
                    perf_mode=mybir.MatmulPerfMode.DoubleRow,
                )

        # --- group max (exact reproduction of the reference per-row max) ---
        tok_t = work.tile([128, NBLK * BD], BF16, tag="tok")
        if ngrp == 1:
            nc.vector.tensor_copy(tok_t[:], t_tile[:, 0, :, :])
        else:
            acc = work.tile([128, NBLK * BD], BF16, tag="tmax_acc")
            nc.scalar.copy(acc[:], t_tile[:, ngrp - 1, :, :])
            for r in range(ngrp - 2, 0, -1):
                nxt = tok_t if r == 1 else work.tile(
                    [128, NBLK * BD], BF16, tag=f"tmax{r}", name=f"tmax{r}"
                )
                nc.vector.tensor_tensor(
                    nxt[:], t_tile[:, r, :, :], acc[:], op=mybir.AluOpType.max
                )
                acc = nxt
            nc.vector.tensor_tensor(
                tok_t[:], t_tile[:, 0, :, :], acc[:], op=mybir.AluOpType.max
            )

        # CLS matmuls: must start the out_ps accumulation group AFTER every
        # other start= in its PSUM bank (start clears has_written bank-wide)
        for k in range(6):
            nc.tensor.matmul(
                out_ps[:],
                qclsT_t[:, k * QPC:(k + 1) * QPC],
                dclsT_t[:, k * BD:(k + 1) * BD],
                start=(k == 0),
                stop=False,
            )
        # --- weighted token sum into out_ps ---
        for b in range(NBLK):
            nc.tensor.matmul(
                out_ps[:],
                sel_t[:, b * QPC:(b + 1) * QPC],
                tok_t[:, b * BD:(b + 1) * BD],
                start=False,
                stop=(b == NBLK - 1),
            )

        outsb = work.tile([QPC, BD], F32, tag="outsb")
        nc.scalar.copy(outsb[:], out_ps[:])
        nc.sync.dma_start(out_d[:], outsb[:])
        if debug_taps:
            nc.sync.dma_start(r0dbg_d[:], r_ts[0][:])
            nc.sync.dma_start(tokdbg_d[:], tok_t[:])
            tsb = work.tile([128, ngrp * NBLK * BD], F32, tag="tsb")
            for r in range(ngrp):
                nc.vector.tensor_copy(
                    tsb[:, r * NBLK * BD:(r + 1) * NBLK * BD], t_tile[:, r, :, :])
            nc.sync.dma_start(tdbg_d[:], tsb[:])

    nc.compile()
    return nc


_NC_CACHE = {}


def _get_nc(gsizes, debug_taps=False):
    key = (gsizes, debug_taps)
    if key not in _NC_CACHE:
        _NC_CACHE[key] = build_nc(gsizes, debug_taps)
    return _NC_CACHE[key]


def _digit_onehot(ids, scale):
    ids = ids.astype(np.int64)
    oh = np.zeros(ids.shape + (KD,), np.float32)
    flat = oh.reshape(-1, KD)
    fid = ids.reshape(-1)
    idx = np.arange(fid.size)
    for t in range(NDIG):
        flat[idx, t * DIG + (fid // (DIG ** t)) % DIG] = scale
    return oh


def _doc_groups(did):
    """Duplicate-rank grouping of doc positions (doc-side data only)."""
    ranks = np.zeros_like(did, dtype=np.int64)
    for d in range(BD):
        seen = {}
        for j in range(LD):
            v = int(did[d, j])
            r = seen.get(v, 0)
            seen[v] = r + 1
            ranks[d, j] = r
    nrank = int(ranks.max()) + 1
    gsizes = []
    pos = []
    for r in range(nrank):
        cnt = (ranks == r).sum(axis=1)
        gr = G0 if r == 0 else max(2, int(np.ceil(cnt.max() / 2)) * 2)
        p = np.full((BD, gr), -1, np.int64)
        for d in range(BD):
            js = np.nonzero(ranks[d] == r)[0]
            p[d, :len(js)] = js
        gsizes.append(gr)
        pos.append(p)
    return tuple(gsizes), pos


def _hilo8(x):
    hi = x.astype(E4)
    lo = (x - hi.astype(np.float32)).astype(E4)
    return hi.astype(np.float32), lo.astype(np.float32)


def make_in_maps(qte, dte, qce, dce, qid, did, qam):
    # SEP mask + CLS drop -> per-token weights
    sep = qam.sum(1) - 1
    qm = qam.astype(np.float32).copy()
    qm[np.arange(BQ), sep] = 0.0
    w = qm.copy()
    w[:, 0] = 0.0

    gsizes, pos = _doc_groups(did)
    ndp = BD * sum(gsizes)
    cb = _chunks(ndp)

    # doc-side feature pairs [KP, ndp, 2] fp8: (d8, d8) / onehot pairs / (1, 1)
    doh = _digit_onehot(did, 1.0)                  # [BD, LD, KD]
    d8 = dte.astype(E4).astype(np.float32)         # [BD, LD, TOK_D]
    dfeat = np.zeros((KP, BD * LD, 2), np.float32)
    d8f = d8.transpose(2, 0, 1).reshape(TOK_D, BD * LD)
    dfeat[0:TOK_D, :, 0] = d8f
    dfeat[0:TOK_D, :, 1] = d8f
    dohf = doh.transpose(2, 0, 1).reshape(KD, BD * LD)
    for j in range(KD // 2):
        dfeat[TOK_D + j, :, 0] = dohf[2 * j]
        dfeat[TOK_D + j, :, 1] = dohf[2 * j + 1]
    dfeat[KP - 1, :, :] = 2.0

    rhs = np.zeros((KP, ndp, 2), E4)
    off = 0
    for r, gr in enumerate(gsizes):
        idx = pos[r]                               # [BD, gr], -1 pad
        src = np.where(idx >= 0, np.arange(BD)[:, None] * LD + np.maximum(idx, 0), 0)
        block = dfeat[:, src.reshape(-1), :].astype(E4)
        block[:, (idx < 0).reshape(-1), :] = 0
        rhs[:, off:off + BD * gr] = block
        off += BD * gr

    qoh = _digit_onehot(qid, C)                    # [BQ, LQ, KD]
    dclsT = np.ascontiguousarray(
        dce.T.reshape(CLS_D // 128, 128, BD)).astype(ml_dtypes.bfloat16)
    idp = np.zeros((128, 2 * 128), dtype=E4)
    for p in range(128):
        idp[p, p] = 1.0
        idp[p, 128 + p] = 1.0

    rhs_chunks = {
        f"rhs{i}": np.ascontiguousarray(
            rhs[:, cb[i]:cb[i + 1], :].reshape(KP, -1))
        for i in range(len(cb) - 1)
    }

    in_maps = []
    for c in range(NCORES):
        qs = slice(c * QPC, (c + 1) * QPC)
        qte_c, qoh_c, w_c = qte[qs], qoh[qs], w[qs]

        qlhsT = np.zeros((NBLK, KP, 2, ROWS), np.float32)
        for b in range(NBLK):
            blk = qte_c[b * 4:(b + 1) * 4].reshape(ROWS, TOK_D)
            qh, ql = _hilo8(blk)
            qlhsT[b, 0:TOK_D, 0] = qh.T
            qlhsT[b, 0:TOK_D, 1] = ql.T
            ohb = qoh_c[b * 4:(b + 1) * 4].reshape(ROWS, KD).T
            for j in range(KD // 2):
                qlhsT[b, TOK_D + j, 0] = ohb[2 * j]
                qlhsT[b, TOK_D + j, 1] = ohb[2 * j + 1]
            qlhsT[b, KP - 1, :, :] = BIAS

        sel = np.zeros((NBLK, ROWS, QPC), np.float32)
        for b in range(NBLK):
            for qq in range(4):
                ql_ = b * 4 + qq
                sel[b, qq * 32:(qq + 1) * 32, ql_] = w_c[ql_]

        qclsT = np.ascontiguousarray(
            qce[qs].T.reshape(CLS_D // 128, 128, QPC)).astype(ml_dtypes.bfloat16)

        im = {
            "qlhsT": qlhsT.reshape(NBLK, KP, 2 * ROWS).astype(E4),
            "sel": sel.astype(ml_dtypes.bfloat16),
            "qclsT": qclsT,
            "dclsT": dclsT,
            "idp": idp,
        }
        im.update(rhs_chunks)
        in_maps.append(im)
    return gsizes, in_maps


def run(gsizes, in_maps, trace=False, debug_taps=False, **kwargs):
    nc = _get_nc(gsizes, debug_taps)
    return run_bass_kernel_spmd(
        nc, in_maps, core_ids=list(range(NCORES)), trace=trace, **kwargs
    )


def kernel(
    query_tok_embs,
    doc_tok_embs,
    query_cls_emb,
    doc_cls_emb,
    query_input_ids,
    doc_input_ids,
    query_attention_mask,
):
    qte = np.ascontiguousarray(np.asarray(query_tok_embs, np.float32))
    dte = np.ascontiguousarray(np.asarray(doc_tok_embs, np.float32))
    qce = np.ascontiguousarray(np.asarray(query_cls_emb, np.float32))
    dce = np.ascontiguousarray(np.asarray(doc_cls_emb, np.float32))
    qid = np.asarray(query_input_ids).astype(np.int64)
    did = np.asarray(doc_input_ids).astype(np.int64)
    qam = np.asarray(query_attention_mask).astype(np.int64)

    gsizes, in_maps = make_in_maps(qte, dte, qce, dce, qid, did, qam)
    res = run(gsizes, in_maps)
    out = np.concatenate([r["out"] for r in res.results], axis=0)
    return np.ascontiguousarray(out.astype(np.float32))


# revision 12
# speedup vs baseline: 1.4750x; 1.1636x over previous
"""COIL-style sparse-attention scoring kernel for Trainium2 (8 NeuronCores).

Reference computation:
    scores[q,i,d,j] = <query_tok_embs[q,i], doc_tok_embs[d,j]>         (K=32)
    masked = where(query_ids[q,i]==doc_ids[d,j], scores, 0)
    tok    = masked.max(axis=j)
    tok_scores[q,d] = sum_i w[q,i] * tok[q,i,d]    (w drops CLS + SEP)
    out = tok_scores + query_cls_emb @ doc_cls_emb.T

Data-parallel over the 64 queries (8 per core, 2 row-blocks of 128 rows =
4 queries x 32 tokens); doc side replicated.

Device algorithm (v3 -- fp8 DoubleRow + sum-decode):

  * The cartesian score+match matmul runs as fp8(e4m3) DoubleRow at 2
    cols/cycle with K=96 packed as 48 partition-pairs:
      pairs 0..31: query (q_hi[e], q_lo[e])  x  doc (d8[e], d8[e])
      pairs 32..46: query 128*onehot pairs   x  doc onehot pairs
      pair  47:    query (-160, -160)        x  doc (2, 2)
    giving  aug = score + 128*(#matching base-6 id digits) - 640  in PSUM.
    A full 5-digit id match makes aug = score; otherwise aug <= score-128.
  * relu(aug) == the where-masked score.  The per-(token,doc) max over doc
    positions j is replaced by a SUM of relu(aug) over j, exact because doc
    positions are permuted host-side (doc data only) into duplicate-rank
    groups: within a group no id repeats inside a doc, so each (token,doc)
    row has at most one match per group.  tok = max over the few groups.
  * relu decode: fp32-PSUM -> fp8-SBUF tensor_scalar/activation split
    across Vector AND Scalar engines (the only PSUM-readers), contiguous
    writes in raw PSUM order.
  * j-sum on TensorE: per phase-pair, one fp8 DoubleRow matmul with
    identity-pair weights accumulates r[.,2p]+r[.,2p+1] over all (row,doc)
    into a [128, 2*128] PSUM tile; the strided rhs AP does the reorder.
  * Group-max on VectorE ([128, 256] tiles), then CLS (bf16 K=768) and the
    weighted token sum (K=128) accumulate into one [8, 128] PSUM tile.
"""

import numpy as np
import ml_dtypes
from contextlib import ExitStack

import concourse.bass as bass
import concourse.bacc as bacc
import concourse.mybir as mybir
import concourse.tile as tile
from concourse.bass_utils import run_bass_kernel_spmd

F32 = mybir.dt.float32
F16 = mybir.dt.float16
BF16 = mybir.dt.bfloat16
FP8 = mybir.dt.float8e4
E4 = ml_dtypes.float8_e4m3

# problem shape (hardcoded per contract)
BQ, LQ, BD, LD, TOK_D, CLS_D = 64, 32, 128, 192, 32, 768
NCORES = 8
QPC = BQ // NCORES          # 8 queries per core
NBLK = 2                    # two row-blocks of 128 = 4 queries x 32 tokens
ROWS = 128
DIG = 6                     # digit base; 6^5 = 7776 > 5000 vocab
NDIG = 5
KD = NDIG * DIG             # 30 one-hot dims
KP = TOK_D + KD // 2 + 1    # 48 partition-pairs (K=96)
C = 128.0                   # per-digit match bonus
OFF = NDIG * C              # 640 full-match offset
BIAS = -OFF / 4             # -160: fp8-exact (|x|<=240), x2 via the pair
                            # and x2 via the doc-side bias value of 2.0
G0 = LD                     # rank-0 group is always padded to LD slots


def _chunks(ndp):
    """rhs DMA chunk boundaries (cols); small leading chunk, 1536-aligned."""
    cb = [0, 1536, 4608]
    while cb[-1] + 4608 < ndp:
        cb.append(cb[-1] + 4608)
    cb.append(ndp)
    return cb


def build_nc(gsizes, debug_taps=False):
    """gsizes: tuple of per-doc group slot counts, gsizes[0] == 192, rest even."""
    assert gsizes[0] == G0 and all(g % 2 == 0 for g in gsizes[1:])
    ngrp = len(gsizes)
    ndp = BD * sum(gsizes)              # doc-position columns per block
    reg_off = np.cumsum([0] + [BD * g for g in gsizes]).tolist()
    cb = _chunks(ndp)

    nc = bacc.Bacc(
        "TRN2",
        target_bir_lowering=False,
        debug=False,
        num_devices=NCORES,
    )

    # qlhsT[b]: [KP, 2, ROWS] fp8 pair-major weights per block
    qlhsT_d = nc.dram_tensor("qlhsT", [NBLK, KP, 2 * ROWS], FP8, kind="ExternalInput")
    # rhs chunk-major: chunk c holds [KP, w_c * 2] fp8 (pair slot innermost)
    rhs_ds = [
        nc.dram_tensor(f"rhs{i}", [KP, (cb[i + 1] - cb[i]) * 2], FP8,
                       kind="ExternalInput")
        for i in range(len(cb) - 1)
    ]
    sel_d = nc.dram_tensor("sel", [NBLK, ROWS, QPC], BF16, kind="ExternalInput")
    qclsT_d = nc.dram_tensor("qclsT", [CLS_D // 128, 128, QPC], BF16, kind="ExternalInput")
    dclsT_d = nc.dram_tensor("dclsT", [CLS_D // 128, 128, BD], BF16, kind="ExternalInput")
    idp_d = nc.dram_tensor("idp", [128, 2 * 128], FP8, kind="ExternalInput")
    out_d = nc.dram_tensor("out", [QPC, BD], F32, kind="ExternalOutput")
    if debug_taps:
        r0dbg_d = nc.dram_tensor("r0dbg", [128, NBLK * BD * gsizes[0]], FP8, kind="ExternalOutput")
        tokdbg_d = nc.dram_tensor("tokdbg", [128, NBLK * BD], BF16, kind="ExternalOutput")
        tdbg_d = nc.dram_tensor("tdbg", [128, ngrp * NBLK * BD], F32, kind="ExternalOutput")

    with tile.TileContext(nc) as tc, ExitStack() as ctx:
        const = ctx.enter_context(tc.tile_pool(name="const", bufs=1))
        psum = ctx.enter_context(tc.tile_pool(name="psum", bufs=2, space="PSUM"))
        tpsum = ctx.enter_context(tc.tile_pool(name="tpsum", bufs=1, space="PSUM"))
        work = ctx.enter_context(tc.tile_pool(name="work", bufs=1))

        # --- SBUF tiles ---
        rhs_t = const.tile([KP, 2 * ndp], FP8, tag="rhs")       # [p, (col, pair)]
        qlhsT_t = const.tile([KP, NBLK * 2 * ROWS], FP8, tag="qlhsT")
        sel_t = const.tile([ROWS, NBLK * QPC], BF16, tag="sel")
        qclsT_t = const.tile([128, 6 * QPC], BF16, tag="qclsT")
        dclsT_t = const.tile([128, 6 * BD], BF16, tag="dclsT")
        idp_t = const.tile([128, 2 * 128], FP8, tag="idp")
        # relu'd decode, raw PSUM column order:
        #   r0[p, (block, group, doc-in-group, phase)]   (phase innermost)
        #   r1+[p, (block, doc, slot)]                   (slot innermost)
        r_ts = []
        for r in range(ngrp):
            r_t = const.tile(
                [128, NBLK * BD * gsizes[r]], FP8, tag=f"r{r}", name=f"r{r}",
            )
            r_ts.append(r_t)

        # --- input DMA: all rhs on the scalar HWDGE queue (the sync queue
        # trickles); small tensors on gpsimd SWDGE ---
        for b in range(NBLK):
            nc.scalar.dma_start(
                qlhsT_t[:, b * 2 * ROWS:(b + 1) * 2 * ROWS], qlhsT_d[b]
            )
        for i in range(len(cb) - 1):
            nc.scalar.dma_start(rhs_t[:, 2 * cb[i]:2 * cb[i + 1]], rhs_ds[i][:])
        nc.gpsimd.dma_start(idp_t[:], idp_d[:])
        for b in range(NBLK):
            nc.gpsimd.dma_start(sel_t[:, b * QPC:(b + 1) * QPC], sel_d[b])
        for k in range(6):
            nc.gpsimd.dma_start(qclsT_t[:, k * QPC:(k + 1) * QPC], qclsT_d[k])
            nc.gpsimd.dma_start(dclsT_t[:, k * BD:(k + 1) * BD], dclsT_d[k])

        # T tile also hosts the [8, 128] output accumulator as slot `ngrp`
        t_tile = tpsum.tile([128, ngrp + 1, NBLK, BD], F32, tag="T")
        out_ps = t_tile[0:QPC, ngrp, 0, :]

        rhs_pairs = rhs_t[:].rearrange("p (n o) -> p o n", o=2)

        def aug_mm(ps_slice, b, c0, cw):
            nc.tensor.matmul(
                ps_slice,
                qlhsT_t[:, b * 2 * ROWS:(b + 1) * 2 * ROWS].rearrange(
                    "p (o m) -> p o m", o=2),
                rhs_pairs[:, :, c0:c0 + cw],
                start=True, stop=True,
                perf_mode=mybir.MatmulPerfMode.DoubleRow,
            )

        # --- region 0: aug matmuls + relu decode, 3-bank groups of 8 docs ---
        dgrp = 3 * 512 // G0                      # 8 docs per 3-bank group
        ng0 = BD // dgrp                          # 16 groups
        ralt = 0
        for g in range(ng0):
            for b in range(NBLK):
                ps = psum.tile([128, 3, 512], F32, tag="aug")
                for k in range(3):
                    aug_mm(ps[:, k, :], b, g * 1536 + k * 512, 512)
                src = ps[:].rearrange("p a t -> p (a t)")
                nslab = dgrp * G0
                dst = r_ts[0][:, (b * ng0 + g) * nslab:(b * ng0 + g + 1) * nslab]
                if ralt % 2 == 0:
                    nc.scalar.activation(dst, src, mybir.ActivationFunctionType.Relu)
                else:
                    nc.vector.tensor_scalar_max(dst, src, 0.0)
                ralt += 1

        # --- regions 1+: small dup-rank groups ---
        for r in range(1, ngrp):
            gr = gsizes[r]
            for b in range(NBLK):
                ps = psum.tile([128, 3, 512], F32, tag="aug")
                pr = ps[:].rearrange("p a t -> p (a t)")[:, 0:BD * gr]
                for c0 in range(0, BD * gr, 512):
                    cw = min(512, BD * gr - c0)
                    aug_mm(pr[:, c0:c0 + cw], b, reg_off[r] + c0, cw)
                dst = r_ts[r][:, b * BD * gr:(b + 1) * BD * gr]
                if r % 2 == 1:
                    nc.scalar.activation(dst, pr, mybir.ActivationFunctionType.Relu)
                else:
                    nc.vector.tensor_scalar_max(dst, pr, 0.0)

        # --- T matmuls: DoubleRow fp8, identity-pair weights, accumulate
        #     phase pairs into T[p, (block, doc)]; strided rhs AP reorders ---
        idp_ap = idp_t[:].rearrange("p (o m) -> p o m", o=2)
        dgrp0 = 3 * 512 // G0
        for r in range(ngrp):
            gr = gsizes[r]
            npp = gr // 2
            for pp in range(npp):
                if r == 0:
                    # r0[p, (b, g, pp, dl, par)] -> [p, par, (b g), dl]
                    rhs_ap = r_ts[0][:].rearrange(
                        "p (bg pp dl par) -> p pp par bg dl",
                        pp=npp, dl=dgrp0, par=2,
                    )[:, pp, :, :, :]
                else:
                    # r[p, (b, pp, d, par)] -> [p, par, b, d]
                    rhs_ap = r_ts[r][:].rearrange(
                        "p (b pp d par) -> p pp par b d",
                        b=NBLK, d=BD, par=2,
                    )[:, pp, :, :, :]
                nc.tensor.matmul(
                    t_tile[:, r, :, :], idp_ap, rhs_ap,
                    start=(pp == 0), stop=(pp == npp - 1),
                    perf_mode=mybir.MatmulPerfMode.DoubleRow,
                )

        # --- group max (exact reproduction of the reference per-row max) ---
        tok_t = work.tile([128, NBLK * BD], BF16, tag="tok")
        if ngrp == 1:
            nc.vector.tensor_copy(tok_t[:], t_tile[:, 0, :, :])
        else:
            acc = work.tile([128, NBLK * BD], BF16, tag="tmax_acc")
            nc.scalar.copy(acc[:], t_tile[:, ngrp - 1, :, :])
            for r in range(ngrp - 2, 0, -1):
                nxt = tok_t if r == 1 else work.tile(
                    [128, NBLK * BD], BF16, tag=f"tmax{r}", name=f"tmax{r}"
                )
                nc.vector.tensor_tensor(
                    nxt[:], t_tile[:, r, :, :], acc[:], op=mybir.AluOpType.max
                )
                acc = nxt
            nc.vector.tensor_tensor(
                tok_t[:], t_tile[:, 0, :, :], acc[:], op=mybir.AluOpType.max
            )

        # CLS matmuls: must start the out_ps accumulation group AFTER every
        # other start= in its PSUM bank (start clears has_written bank-wide)
        for k in range(6):
            nc.tensor.matmul(
                out_ps[:],
                qclsT_t[:, k * QPC:(k + 1) * QPC],
                dclsT_t[:, k * BD:(k + 1) * BD],
                start=(k == 0),
                stop=False,
            )
        # --- weighted token sum into out_ps ---
        for b in range(NBLK):
            nc.tensor.matmul(
                out_ps[:],
                sel_t[:, b * QPC:(b + 1) * QPC],
                tok_t[:, b * BD:(b + 1) * BD],
                start=False,
                stop=(b == NBLK - 1),
            )

        outsb = work.tile([QPC, BD], F32, tag="outsb")
        nc.scalar.copy(outsb[:], out_ps[:])
        nc.sync.dma_start(out_d[:], outsb[:])
        if debug_taps:
            nc.sync.dma_start(r0dbg_d[:], r_ts[0][:])
            nc.sync.dma_start(tokdbg_d[:], tok_t[:])
            tsb = work.tile([128, ngrp * NBLK * BD], F32, tag="tsb")
            for r in range(ngrp):
                nc.vector.tensor_copy(
                    tsb[:, r * NBLK * BD:(r + 1) * NBLK * BD], t_tile[:, r, :, :])
            nc.sync.dma_start(tdbg_d[:], tsb[:])

    nc.compile()
    return nc


_NC_CACHE = {}


def _get_nc(gsizes, debug_taps=False):
    key = (gsizes, debug_taps)
    if key not in _NC_CACHE:
        _NC_CACHE[key] = build_nc(gsizes, debug_taps)
    return _NC_CACHE[key]


def _digit_onehot(ids, scale):
    ids = ids.astype(np.int64)
    oh = np.zeros(ids.shape + (KD,), np.float32)
    flat = oh.reshape(-1, KD)
    fid = ids.reshape(-1)
    idx = np.arange(fid.size)
    for t in range(NDIG):
        flat[idx, t * DIG + (fid // (DIG ** t)) % DIG] = scale
    return oh


def _doc_groups(did):
    """Duplicate-rank grouping of doc positions (doc-side data only)."""
    ranks = np.zeros_like(did, dtype=np.int64)
    for d in range(BD):
        seen = {}
        for j in range(LD):
            v = int(did[d, j])
            r = seen.get(v, 0)
            seen[v] = r + 1
            ranks[d, j] = r
    nrank = int(ranks.max()) + 1
    gsizes = []
    pos = []
    for r in range(nrank):
        cnt = (ranks == r).sum(axis=1)
        gr = G0 if r == 0 else max(2, int(np.ceil(cnt.max() / 2)) * 2)
        p = np.full((BD, gr), -1, np.int64)
        for d in range(BD):
            js = np.nonzero(ranks[d] == r)[0]
            p[d, :len(js)] = js
        gsizes.append(gr)
        pos.append(p)
    return tuple(gsizes), pos


def _hilo8(x):
    hi = x.astype(E4)
    lo = (x - hi.astype(np.float32)).astype(E4)
    return hi.astype(np.float32), lo.astype(np.float32)


def make_in_maps(qte, dte, qce, dce, qid, did, qam):
    # SEP mask + CLS drop -> per-token weights
    sep = qam.sum(1) - 1
    qm = qam.astype(np.float32).copy()
    qm[np.arange(BQ), sep] = 0.0
    w = qm.copy()
    w[:, 0] = 0.0

    gsizes, pos = _doc_groups(did)
    ndp = BD * sum(gsizes)
    cb = _chunks(ndp)

    # doc-side feature pairs [KP, ndp, 2] fp8: (d8, d8) / onehot pairs / (1, 1)
    doh = _digit_onehot(did, 1.0)                  # [BD, LD, KD]
    d8 = dte.astype(E4).astype(np.float32)         # [BD, LD, TOK_D]
    dfeat = np.zeros((KP, BD * LD, 2), np.float32)
    d8f = d8.transpose(2, 0, 1).reshape(TOK_D, BD * LD)
    dfeat[0:TOK_D, :, 0] = d8f
    dfeat[0:TOK_D, :, 1] = d8f
    dohf = doh.transpose(2, 0, 1).reshape(KD, BD * LD)
    for j in range(KD // 2):
        dfeat[TOK_D + j, :, 0] = dohf[2 * j]
        dfeat[TOK_D + j, :, 1] = dohf[2 * j + 1]
    dfeat[KP - 1, :, :] = 2.0

    rhs = np.zeros((KP, ndp, 2), E4)
    off = 0
    dgrp0 = 8
    for r, gr in enumerate(gsizes):
        idx = pos[r]                               # [BD, gr], -1 pad
        if r == 0:
            # column order (g, pp, dl, par): doc = 8g+dl, slot = 2pp+par
            iv = idx.reshape(BD // dgrp0, dgrp0, gr // 2, 2)   # [g, dl, pp, par]
            iv = iv.transpose(0, 2, 1, 3)                      # [g, pp, dl, par]
        else:
            # column order (pp, d, par)
            iv = idx.reshape(BD, gr // 2, 2)                   # [d, pp, par]
            iv = iv.transpose(1, 0, 2)                         # [pp, d, par]
        iv = np.ascontiguousarray(iv)
        docof = {0: (np.arange(BD).reshape(BD // dgrp0, dgrp0, 1, 1)
                     .transpose(0, 2, 1, 3) if False else None)}
        # doc index per column, matching iv's layout
        if r == 0:
            dv = np.broadcast_to(
                np.arange(BD).reshape(BD // dgrp0, 1, dgrp0, 1),
                iv.shape)
        else:
            dv = np.broadcast_to(np.arange(BD).reshape(1, BD, 1), iv.shape)
        flat_i = iv.reshape(-1)
        flat_d = dv.reshape(-1)
        src = np.where(flat_i >= 0, flat_d * LD + np.maximum(flat_i, 0), 0)
        block = dfeat[:, src, :].astype(E4)
        block[:, flat_i < 0, :] = 0
        rhs[:, off:off + BD * gr] = block
        off += BD * gr

    qoh = _digit_onehot(qid, C)                    # [BQ, LQ, KD]
    dclsT = np.ascontiguousarray(
        dce.T.reshape(CLS_D // 128, 128, BD)).astype(ml_dtypes.bfloat16)
    idp = np.zeros((128, 2 * 128), dtype=E4)
    for p in range(128):
        idp[p, p] = 1.0
        idp[p, 128 + p] = 1.0

    rhs_chunks = {
        f"rhs{i}": np.ascontiguousarray(
            rhs[:, cb[i]:cb[i + 1], :].reshape(KP, -1))
        for i in range(len(cb) - 1)
    }

    in_maps = []
    for c in range(NCORES):
        qs = slice(c * QPC, (c + 1) * QPC)
        qte_c, qoh_c, w_c = qte[qs], qoh[qs], w[qs]

        qlhsT = np.zeros((NBLK, KP, 2, ROWS), np.float32)
        for b in range(NBLK):
            blk = qte_c[b * 4:(b + 1) * 4].reshape(ROWS, TOK_D)
            qh, ql = _hilo8(blk)
            qlhsT[b, 0:TOK_D, 0] = qh.T
            qlhsT[b, 0:TOK_D, 1] = ql.T
            ohb = qoh_c[b * 4:(b + 1) * 4].reshape(ROWS, KD).T
            for j in range(KD // 2):
                qlhsT[b, TOK_D + j, 0] = ohb[2 * j]
                qlhsT[b, TOK_D + j, 1] = ohb[2 * j + 1]
            qlhsT[b, KP - 1, :, :] = BIAS

        sel = np.zeros((NBLK, ROWS, QPC), np.float32)
        for b in range(NBLK):
            for qq in range(4):
                ql_ = b * 4 + qq
                sel[b, qq * 32:(qq + 1) * 32, ql_] = w_c[ql_]

        qclsT = np.ascontiguousarray(
            qce[qs].T.reshape(CLS_D // 128, 128, QPC)).astype(ml_dtypes.bfloat16)

        im = {
            "qlhsT": qlhsT.reshape(NBLK, KP, 2 * ROWS).astype(E4),
            "sel": sel.astype(ml_dtypes.bfloat16),
            "qclsT": qclsT,
            "dclsT": dclsT,
            "idp": idp,
        }
        im.update(rhs_chunks)
        in_maps.append(im)
    return gsizes, in_maps


def run(gsizes, in_maps, trace=False, debug_taps=False, **kwargs):
    nc = _get_nc(gsizes, debug_taps)
    return run_bass_kernel_spmd(
        nc, in_maps, core_ids=list(range(NCORES)), trace=trace, **kwargs
    )


def kernel(
    query_tok_embs,
    doc_tok_embs,
    query_cls_emb,
    doc_cls_emb,
    query_input_ids,
    doc_input_ids,
    query_attention_mask,
):
    qte = np.ascontiguousarray(np.asarray(query_tok_embs, np.float32))
    dte = np.ascontiguousarray(np.asarray(doc_tok_embs, np.float32))
    qce = np.ascontiguousarray(np.asarray(query_cls_emb, np.float32))
    dce = np.ascontiguousarray(np.asarray(doc_cls_emb, np.float32))
    qid = np.asarray(query_input_ids).astype(np.int64)
    did = np.asarray(doc_input_ids).astype(np.int64)
    qam = np.asarray(query_attention_mask).astype(np.int64)

    gsizes, in_maps = make_in_maps(qte, dte, qce, dce, qid, did, qam)
    res = run(gsizes, in_maps)
    out = np.concatenate([r["out"] for r in res.results], axis=0)
    return np.ascontiguousarray(out.astype(np.float32))


# revision 13
# speedup vs baseline: 1.7226x; 1.1679x over previous
"""COIL-style sparse-attention scoring kernel for Trainium2 (8 NeuronCores).

Reference computation:
    scores[q,i,d,j] = <query_tok_embs[q,i], doc_tok_embs[d,j]>         (K=32)
    masked = where(query_ids[q,i]==doc_ids[d,j], scores, 0)
    tok    = masked.max(axis=j)
    tok_scores[q,d] = sum_i w[q,i] * tok[q,i,d]    (w drops CLS + SEP)
    out = tok_scores + query_cls_emb @ doc_cls_emb.T

Data-parallel over the 64 queries (8 per core, 2 row-blocks of 128 rows =
4 queries x 32 tokens); doc side replicated.

Device algorithm (v3 -- fp8 DoubleRow + sum-decode):

  * The cartesian score+match matmul runs as fp8(e4m3) DoubleRow at 2
    cols/cycle with K=96 packed as 48 partition-pairs:
      pairs 0..31: query (q_hi[e], q_lo[e])  x  doc (d8[e], d8[e])
      pairs 32..46: query 128*onehot pairs   x  doc onehot pairs
      pair  47:    query (-160, -160)        x  doc (2, 2)
    giving  aug = score + 128*(#matching base-6 id digits) - 640  in PSUM.
    A full 5-digit id match makes aug = score; otherwise aug <= score-128.
  * relu(aug) == the where-masked score.  The per-(token,doc) max over doc
    positions j is replaced by a SUM of relu(aug) over j, exact because doc
    positions are permuted host-side (doc data only) into duplicate-rank
    groups: within a group no id repeats inside a doc, so each (token,doc)
    row has at most one match per group.  tok = max over the few groups.
  * relu decode: fp32-PSUM -> fp8-SBUF tensor_scalar/activation split
    across Vector AND Scalar engines (the only PSUM-readers), contiguous
    writes in raw PSUM order.
  * j-sum on TensorE: per phase-pair, one fp8 DoubleRow matmul with
    identity-pair weights accumulates r[.,2p]+r[.,2p+1] over all (row,doc)
    into a [128, 2*128] PSUM tile; the strided rhs AP does the reorder.
  * Group-max on VectorE ([128, 256] tiles), then CLS (bf16 K=768) and the
    weighted token sum (K=128) accumulate into one [8, 128] PSUM tile.
"""

import numpy as np
import ml_dtypes
from contextlib import ExitStack

import concourse.bass as bass
import concourse.bacc as bacc
import concourse.mybir as mybir
import concourse.tile as tile
from concourse.bass_utils import run_bass_kernel_spmd

F32 = mybir.dt.float32
F16 = mybir.dt.float16
BF16 = mybir.dt.bfloat16
FP8 = mybir.dt.float8e4
E4 = ml_dtypes.float8_e4m3

# problem shape (hardcoded per contract)
BQ, LQ, BD, LD, TOK_D, CLS_D = 64, 32, 128, 192, 32, 768
NCORES = 8
QPC = BQ // NCORES          # 8 queries per core
NBLK = 2                    # two row-blocks of 128 = 4 queries x 32 tokens
ROWS = 128
DIG = 6                     # digit base; 6^5 = 7776 > 5000 vocab
NDIG = 5
KD = NDIG * DIG             # 30 one-hot dims
KP = TOK_D + KD // 2 + 1    # 48 partition-pairs (K=96)
C = 128.0                   # per-digit match bonus
OFF = NDIG * C              # 640 full-match offset
BIAS = -OFF / 4             # -160: fp8-exact (|x|<=240), x2 via the pair
                            # and x2 via the doc-side bias value of 2.0
G0 = LD                     # rank-0 group is always padded to LD slots


def _chunks(ndp):
    """rhs DMA chunk boundaries (cols); small leading chunk, 1536-aligned."""
    cb = [0, 1536, 4608]
    while cb[-1] + 4608 < ndp:
        cb.append(cb[-1] + 4608)
    cb.append(ndp)
    return cb


def build_nc(gsizes, debug_taps=False):
    """gsizes: tuple of per-doc group slot counts, gsizes[0] == 192, rest even."""
    assert gsizes[0] == G0 and all(g % 2 == 0 for g in gsizes[1:])
    ngrp = len(gsizes)
    ndp = BD * sum(gsizes)              # doc-position columns per block
    reg_off = np.cumsum([0] + [BD * g for g in gsizes]).tolist()
    cb = _chunks(ndp)

    nc = bacc.Bacc(
        "TRN2",
        target_bir_lowering=False,
        debug=False,
        num_devices=NCORES,
    )

    # qlhsT[b]: [KP, 2, ROWS] fp8 pair-major weights per block
    qlhsT_d = nc.dram_tensor("qlhsT", [NBLK, KP, 2 * ROWS], FP8, kind="ExternalInput")
    # rhs chunk-major: chunk c holds [KP, w_c * 2] fp8 (pair slot innermost)
    rhs_ds = [
        nc.dram_tensor(f"rhs{i}", [KP, (cb[i + 1] - cb[i]) * 2], FP8,
                       kind="ExternalInput")
        for i in range(len(cb) - 1)
    ]
    sel_d = nc.dram_tensor("sel", [NBLK, ROWS, QPC], BF16, kind="ExternalInput")
    qclsT_d = nc.dram_tensor("qclsT", [CLS_D // 128, 128, QPC], BF16, kind="ExternalInput")
    dclsT_d = nc.dram_tensor("dclsT", [CLS_D // 128, 128, BD], BF16, kind="ExternalInput")
    idp_d = nc.dram_tensor("idp", [128, 2 * 128], FP8, kind="ExternalInput")
    out_d = nc.dram_tensor("out", [QPC, BD], F32, kind="ExternalOutput")
    if debug_taps:
        r0dbg_d = nc.dram_tensor("r0dbg", [128, NBLK * BD * gsizes[0]], FP8, kind="ExternalOutput")
        tokdbg_d = nc.dram_tensor("tokdbg", [128, NBLK * BD], BF16, kind="ExternalOutput")
        tdbg_d = nc.dram_tensor("tdbg", [128, ngrp * NBLK * BD], F32, kind="ExternalOutput")

    with tile.TileContext(nc) as tc, ExitStack() as ctx:
        const = ctx.enter_context(tc.tile_pool(name="const", bufs=1))
        psum = ctx.enter_context(tc.tile_pool(name="psum", bufs=2, space="PSUM"))
        tpsum = ctx.enter_context(tc.tile_pool(name="tpsum", bufs=1, space="PSUM"))
        work = ctx.enter_context(tc.tile_pool(name="work", bufs=1))

        # --- SBUF tiles ---
        rhs_t = const.tile([KP, 2 * ndp], FP8, tag="rhs")       # [p, (col, pair)]
        qlhsT_t = const.tile([KP, NBLK * 2 * ROWS], FP8, tag="qlhsT")
        sel_t = const.tile([ROWS, NBLK * QPC], BF16, tag="sel")
        qclsT_t = const.tile([128, 6 * QPC], BF16, tag="qclsT")
        dclsT_t = const.tile([128, 6 * BD], BF16, tag="dclsT")
        idp_t = const.tile([128, 2 * 128], FP8, tag="idp")
        # relu'd decode, raw PSUM column order:
        #   r0[p, (block, group, doc-in-group, phase)]   (phase innermost)
        #   r1+[p, (block, doc, slot)]                   (slot innermost)
        r_ts = []
        for r in range(ngrp):
            r_t = const.tile(
                [128, NBLK * BD * gsizes[r]], FP8, tag=f"r{r}", name=f"r{r}",
            )
            r_ts.append(r_t)

        # --- input DMA: all rhs on the scalar HWDGE queue (the sync queue
        # trickles); small tensors on gpsimd SWDGE ---
        for b in range(NBLK):
            nc.scalar.dma_start(
                qlhsT_t[:, b * 2 * ROWS:(b + 1) * 2 * ROWS], qlhsT_d[b]
            )
        for i in range(len(cb) - 1):
            nc.scalar.dma_start(rhs_t[:, 2 * cb[i]:2 * cb[i + 1]], rhs_ds[i][:])
        nc.gpsimd.dma_start(idp_t[:], idp_d[:])
        for b in range(NBLK):
            nc.gpsimd.dma_start(sel_t[:, b * QPC:(b + 1) * QPC], sel_d[b])
        for k in range(6):
            nc.gpsimd.dma_start(qclsT_t[:, k * QPC:(k + 1) * QPC], qclsT_d[k])
            nc.gpsimd.dma_start(dclsT_t[:, k * BD:(k + 1) * BD], dclsT_d[k])

        # T tile also hosts the [8, 128] output accumulator as slot `ngrp`
        t_tile = tpsum.tile([128, ngrp + 1, NBLK, BD], F32, tag="T")
        out_ps = t_tile[0:QPC, ngrp, 0, :]

        rhs_pairs = rhs_t[:].rearrange("p (n o) -> p o n", o=2)

        def aug_mm(ps_slice, b, c0, cw):
            nc.tensor.matmul(
                ps_slice,
                qlhsT_t[:, b * 2 * ROWS:(b + 1) * 2 * ROWS].rearrange(
                    "p (o m) -> p o m", o=2),
                rhs_pairs[:, :, c0:c0 + cw],
                start=True, stop=True,
                perf_mode=mybir.MatmulPerfMode.DoubleRow,
            )

        # --- aug matmuls + relu decode; all of block 0 first, then block 1,
        # so the stationary weights change only once (DR LDWEIGHTS is not
        # hidden when weights alternate) ---
        dgrp = 3 * 512 // G0                      # 8 docs per 3-bank group
        ng0 = BD // dgrp                          # 16 groups
        ralt = 0
        for b in range(NBLK):
            for g in range(ng0):
                ps = psum.tile([128, 3, 512], F32, tag="aug")
                for k in range(3):
                    aug_mm(ps[:, k, :], b, g * 1536 + k * 512, 512)
                src = ps[:].rearrange("p a t -> p (a t)")
                nslab = dgrp * G0
                dst = r_ts[0][:, (b * ng0 + g) * nslab:(b * ng0 + g + 1) * nslab]
                if ralt % 2 == 0:
                    nc.scalar.activation(dst, src, mybir.ActivationFunctionType.Relu)
                else:
                    nc.vector.tensor_scalar_max(dst, src, 0.0)
                ralt += 1
            for r in range(1, ngrp):
                gr = gsizes[r]
                ps = psum.tile([128, 3, 512], F32, tag="aug")
                pr = ps[:].rearrange("p a t -> p (a t)")[:, 0:BD * gr]
                for c0 in range(0, BD * gr, 512):
                    cw = min(512, BD * gr - c0)
                    aug_mm(pr[:, c0:c0 + cw], b, reg_off[r] + c0, cw)
                dst = r_ts[r][:, b * BD * gr:(b + 1) * BD * gr]
                if r % 2 == 1:
                    nc.scalar.activation(dst, pr, mybir.ActivationFunctionType.Relu)
                else:
                    nc.vector.tensor_scalar_max(dst, pr, 0.0)

        # --- T matmuls: DoubleRow fp8, identity-pair weights, accumulate
        #     phase pairs into T[p, (block, doc)]; strided rhs AP reorders ---
        idp_ap = idp_t[:].rearrange("p (o m) -> p o m", o=2)
        dgrp0 = 3 * 512 // G0
        for r in range(ngrp):
            gr = gsizes[r]
            npp = gr // 2
            for pp in range(npp):
                if r == 0:
                    # r0[p, (b, g, pp, dl, par)] -> [p, par, (b g), dl]
                    rhs_ap = r_ts[0][:].rearrange(
                        "p (bg pp dl par) -> p pp par bg dl",
                        pp=npp, dl=dgrp0, par=2,
                    )[:, pp, :, :, :]
                else:
                    # r[p, (b, pp, d, par)] -> [p, par, b, d]
                    rhs_ap = r_ts[r][:].rearrange(
                        "p (b pp d par) -> p pp par b d",
                        b=NBLK, d=BD, par=2,
                    )[:, pp, :, :, :]
                nc.tensor.matmul(
                    t_tile[:, r, :, :], idp_ap, rhs_ap,
                    start=(pp == 0), stop=(pp == npp - 1),
                    perf_mode=mybir.MatmulPerfMode.DoubleRow,
                )

        # --- group max (exact reproduction of the reference per-row max) ---
        tok_t = work.tile([128, NBLK * BD], BF16, tag="tok")
        if ngrp == 1:
            nc.vector.tensor_copy(tok_t[:], t_tile[:, 0, :, :])
        else:
            acc = work.tile([128, NBLK * BD], BF16, tag="tmax_acc")
            nc.scalar.copy(acc[:], t_tile[:, ngrp - 1, :, :])
            for r in range(ngrp - 2, 0, -1):
                nxt = tok_t if r == 1 else work.tile(
                    [128, NBLK * BD], BF16, tag=f"tmax{r}", name=f"tmax{r}"
                )
                nc.vector.tensor_tensor(
                    nxt[:], t_tile[:, r, :, :], acc[:], op=mybir.AluOpType.max
                )
                acc = nxt
            nc.vector.tensor_tensor(
                tok_t[:], t_tile[:, 0, :, :], acc[:], op=mybir.AluOpType.max
            )

        # CLS matmuls: must start the out_ps accumulation group AFTER every
        # other start= in its PSUM bank (start clears has_written bank-wide)
        for k in range(6):
            nc.tensor.matmul(
                out_ps[:],
                qclsT_t[:, k * QPC:(k + 1) * QPC],
                dclsT_t[:, k * BD:(k + 1) * BD],
                start=(k == 0),
                stop=False,
            )
        # --- weighted token sum into out_ps ---
        for b in range(NBLK):
            nc.tensor.matmul(
                out_ps[:],
                sel_t[:, b * QPC:(b + 1) * QPC],
                tok_t[:, b * BD:(b + 1) * BD],
                start=False,
                stop=(b == NBLK - 1),
            )

        outsb = work.tile([QPC, BD], F32, tag="outsb")
        nc.scalar.copy(outsb[:], out_ps[:])
        nc.sync.dma_start(out_d[:], outsb[:])
        if debug_taps:
            nc.sync.dma_start(r0dbg_d[:], r_ts[0][:])
            nc.sync.dma_start(tokdbg_d[:], tok_t[:])
            tsb = work.tile([128, ngrp * NBLK * BD], F32, tag="tsb")
            for r in range(ngrp):
                nc.vector.tensor_copy(
                    tsb[:, r * NBLK * BD:(r + 1) * NBLK * BD], t_tile[:, r, :, :])
            nc.sync.dma_start(tdbg_d[:], tsb[:])

    nc.compile()
    return nc


_NC_CACHE = {}


def _get_nc(gsizes, debug_taps=False):
    key = (gsizes, debug_taps)
    if key not in _NC_CACHE:
        _NC_CACHE[key] = build_nc(gsizes, debug_taps)
    return _NC_CACHE[key]


def _digit_onehot(ids, scale):
    ids = ids.astype(np.int64)
    oh = np.zeros(ids.shape + (KD,), np.float32)
    flat = oh.reshape(-1, KD)
    fid = ids.reshape(-1)
    idx = np.arange(fid.size)
    for t in range(NDIG):
        flat[idx, t * DIG + (fid // (DIG ** t)) % DIG] = scale
    return oh


def _doc_groups(did):
    """Duplicate-rank grouping of doc positions (doc-side data only)."""
    ranks = np.zeros_like(did, dtype=np.int64)
    for d in range(BD):
        seen = {}
        for j in range(LD):
            v = int(did[d, j])
            r = seen.get(v, 0)
            seen[v] = r + 1
            ranks[d, j] = r
    nrank = int(ranks.max()) + 1
    gsizes = []
    pos = []
    for r in range(nrank):
        cnt = (ranks == r).sum(axis=1)
        gr = G0 if r == 0 else max(2, int(np.ceil(cnt.max() / 2)) * 2)
        p = np.full((BD, gr), -1, np.int64)
        for d in range(BD):
            js = np.nonzero(ranks[d] == r)[0]
            p[d, :len(js)] = js
        gsizes.append(gr)
        pos.append(p)
    return tuple(gsizes), pos


def _hilo8(x):
    hi = x.astype(E4)
    lo = (x - hi.astype(np.float32)).astype(E4)
    return hi.astype(np.float32), lo.astype(np.float32)


def make_in_maps(qte, dte, qce, dce, qid, did, qam):
    # SEP mask + CLS drop -> per-token weights
    sep = qam.sum(1) - 1
    qm = qam.astype(np.float32).copy()
    qm[np.arange(BQ), sep] = 0.0
    w = qm.copy()
    w[:, 0] = 0.0

    gsizes, pos = _doc_groups(did)
    ndp = BD * sum(gsizes)
    cb = _chunks(ndp)

    # doc-side feature pairs [KP, ndp, 2] fp8: (d8, d8) / onehot pairs / (1, 1)
    doh = _digit_onehot(did, 1.0)                  # [BD, LD, KD]
    d8 = dte.astype(E4).astype(np.float32)         # [BD, LD, TOK_D]
    dfeat = np.zeros((KP, BD * LD, 2), np.float32)
    d8f = d8.transpose(2, 0, 1).reshape(TOK_D, BD * LD)
    dfeat[0:TOK_D, :, 0] = d8f
    dfeat[0:TOK_D, :, 1] = d8f
    dohf = doh.transpose(2, 0, 1).reshape(KD, BD * LD)
    for j in range(KD // 2):
        dfeat[TOK_D + j, :, 0] = dohf[2 * j]
        dfeat[TOK_D + j, :, 1] = dohf[2 * j + 1]
    dfeat[KP - 1, :, :] = 2.0

    rhs = np.zeros((KP, ndp, 2), E4)
    off = 0
    dgrp0 = 8
    for r, gr in enumerate(gsizes):
        idx = pos[r]                               # [BD, gr], -1 pad
        if r == 0:
            # column order (g, pp, dl, par): doc = 8g+dl, slot = 2pp+par
            iv = idx.reshape(BD // dgrp0, dgrp0, gr // 2, 2)   # [g, dl, pp, par]
            iv = iv.transpose(0, 2, 1, 3)                      # [g, pp, dl, par]
        else:
            # column order (pp, d, par)
            iv = idx.reshape(BD, gr // 2, 2)                   # [d, pp, par]
            iv = iv.transpose(1, 0, 2)                         # [pp, d, par]
        iv = np.ascontiguousarray(iv)
        docof = {0: (np.arange(BD).reshape(BD // dgrp0, dgrp0, 1, 1)
                     .transpose(0, 2, 1, 3) if False else None)}
        # doc index per column, matching iv's layout
        if r == 0:
            dv = np.broadcast_to(
                np.arange(BD).reshape(BD // dgrp0, 1, dgrp0, 1),
                iv.shape)
        else:
            dv = np.broadcast_to(np.arange(BD).reshape(1, BD, 1), iv.shape)
        flat_i = iv.reshape(-1)
        flat_d = dv.reshape(-1)
        src = np.where(flat_i >= 0, flat_d * LD + np.maximum(flat_i, 0), 0)
        block = dfeat[:, src, :].astype(E4)
        block[:, flat_i < 0, :] = 0
        rhs[:, off:off + BD * gr] = block
        off += BD * gr

    qoh = _digit_onehot(qid, C)                    # [BQ, LQ, KD]
    dclsT = np.ascontiguousarray(
        dce.T.reshape(CLS_D // 128, 128, BD)).astype(ml_dtypes.bfloat16)
    idp = np.zeros((128, 2 * 128), dtype=E4)
    for p in range(128):
        idp[p, p] = 1.0
        idp[p, 128 + p] = 1.0

    rhs_chunks = {
        f"rhs{i}": np.ascontiguousarray(
            rhs[:, cb[i]:cb[i + 1], :].reshape(KP, -1))
        for i in range(len(cb) - 1)
    }

    in_maps = []
    for c in range(NCORES):
        qs = slice(c * QPC, (c + 1) * QPC)
        qte_c, qoh_c, w_c = qte[qs], qoh[qs], w[qs]

        qlhsT = np.zeros((NBLK, KP, 2, ROWS), np.float32)
        for b in range(NBLK):
            blk = qte_c[b * 4:(b + 1) * 4].reshape(ROWS, TOK_D)
            qh, ql = _hilo8(blk)
            qlhsT[b, 0:TOK_D, 0] = qh.T
            qlhsT[b, 0:TOK_D, 1] = ql.T
            ohb = qoh_c[b * 4:(b + 1) * 4].reshape(ROWS, KD).T
            for j in range(KD // 2):
                qlhsT[b, TOK_D + j, 0] = ohb[2 * j]
                qlhsT[b, TOK_D + j, 1] = ohb[2 * j + 1]
            qlhsT[b, KP - 1, :, :] = BIAS

        sel = np.zeros((NBLK, ROWS, QPC), np.float32)
        for b in range(NBLK):
            for qq in range(4):
                ql_ = b * 4 + qq
                sel[b, qq * 32:(qq + 1) * 32, ql_] = w_c[ql_]

        qclsT = np.ascontiguousarray(
            qce[qs].T.reshape(CLS_D // 128, 128, QPC)).astype(ml_dtypes.bfloat16)

        im = {
            "qlhsT": qlhsT.reshape(NBLK, KP, 2 * ROWS).astype(E4),
            "sel": sel.astype(ml_dtypes.bfloat16),
            "qclsT": qclsT,
            "dclsT": dclsT,
            "idp": idp,
        }
        im.update(rhs_chunks)
        in_maps.append(im)
    return gsizes, in_maps


def run(gsizes, in_maps, trace=False, debug_taps=False, **kwargs):
    nc = _get_nc(gsizes, debug_taps)
    return run_bass_kernel_spmd(
        nc, in_maps, core_ids=list(range(NCORES)), trace=trace, **kwargs
    )


def kernel(
    query_tok_embs,
    doc_tok_embs,
    query_cls_emb,
    doc_cls_emb,
    query_input_ids,
    doc_input_ids,
    query_attention_mask,
):
    qte = np.ascontiguousarray(np.asarray(query_tok_embs, np.float32))
    dte = np.ascontiguousarray(np.asarray(doc_tok_embs, np.float32))
    qce = np.ascontiguousarray(np.asarray(query_cls_emb, np.float32))
    dce = np.ascontiguousarray(np.asarray(doc_cls_emb, np.float32))
    qid = np.asarray(query_input_ids).astype(np.int64)
    did = np.asarray(doc_input_ids).astype(np.int64)
    qam = np.asarray(query_attention_mask).astype(np.int64)

    gsizes, in_maps = make_in_maps(qte, dte, qce, dce, qid, did, qam)
    res = run(gsizes, in_maps)
    out = np.concatenate([r["out"] for r in res.results], axis=0)
    return np.ascontiguousarray(out.astype(np.float32))


# revision 15
# speedup vs baseline: 1.7412x; 1.0108x over previous
"""COIL-style sparse-attention scoring kernel for Trainium2 (8 NeuronCores).

Reference computation:
    scores[q,i,d,j] = <query_tok_embs[q,i], doc_tok_embs[d,j]>         (K=32)
    masked = where(query_ids[q,i]==doc_ids[d,j], scores, 0)
    tok    = masked.max(axis=j)
    tok_scores[q,d] = sum_i w[q,i] * tok[q,i,d]    (w drops CLS + SEP)
    out = tok_scores + query_cls_emb @ doc_cls_emb.T

Data-parallel over the 64 queries (8 per core, 2 row-blocks of 128 rows =
4 queries x 32 tokens); doc side replicated.

Device algorithm (v3 -- fp8 DoubleRow + sum-decode):

  * The cartesian score+match matmul runs as fp8(e4m3) DoubleRow at 2
    cols/cycle with K=96 packed as 48 partition-pairs:
      pairs 0..31: query (q_hi[e], q_lo[e])  x  doc (d8[e], d8[e])
      pairs 32..46: query 128*onehot pairs   x  doc onehot pairs
      pair  47:    query (-160, -160)        x  doc (2, 2)
    giving  aug = score + 128*(#matching base-6 id digits) - 640  in PSUM.
    A full 5-digit id match makes aug = score; otherwise aug <= score-128.
  * relu(aug) == the where-masked score.  The per-(token,doc) max over doc
    positions j is replaced by a SUM of relu(aug) over j, exact because doc
    positions are permuted host-side (doc data only) into duplicate-rank
    groups: within a group no id repeats inside a doc, so each (token,doc)
    row has at most one match per group.  tok = max over the few groups.
  * relu decode: fp32-PSUM -> fp8-SBUF tensor_scalar/activation split
    across Vector AND Scalar engines (the only PSUM-readers), contiguous
    writes in raw PSUM order.
  * j-sum on TensorE: per phase-pair, one fp8 DoubleRow matmul with
    identity-pair weights accumulates r[.,2p]+r[.,2p+1] over all (row,doc)
    into a [128, 2*128] PSUM tile; the strided rhs AP does the reorder.
  * Group-max on VectorE ([128, 256] tiles), then CLS (bf16 K=768) and the
    weighted token sum (K=128) accumulate into one [8, 128] PSUM tile.
"""

import numpy as np
import ml_dtypes
from contextlib import ExitStack

import concourse.bass as bass
import concourse.bacc as bacc
import concourse.mybir as mybir
import concourse.tile as tile
from concourse.bass_utils import run_bass_kernel_spmd

F32 = mybir.dt.float32
F16 = mybir.dt.float16
BF16 = mybir.dt.bfloat16
FP8 = mybir.dt.float8e4
E4 = ml_dtypes.float8_e4m3

# problem shape (hardcoded per contract)
BQ, LQ, BD, LD, TOK_D, CLS_D = 64, 32, 128, 192, 32, 768
NCORES = 8
QPC = BQ // NCORES          # 8 queries per core
NBLK = 2                    # two row-blocks of 128 = 4 queries x 32 tokens
ROWS = 128
DIG = 6                     # digit base; 6^5 = 7776 > 5000 vocab
NDIG = 5
KD = NDIG * DIG             # 30 one-hot dims
KP = TOK_D + KD // 2 + 1    # 48 partition-pairs (K=96)
C = 128.0                   # per-digit match bonus
OFF = NDIG * C              # 640 full-match offset
BIAS = -OFF / 4             # -160: fp8-exact (|x|<=240), x2 via the pair
                            # and x2 via the doc-side bias value of 2.0
G0 = LD                     # rank-0 group is always padded to LD slots


def _chunks(ndp):
    """rhs DMA chunk boundaries (cols); small leading chunk, 1536-aligned."""
    cb = [0, 1536, 4608]
    while cb[-1] + 4608 < ndp:
        cb.append(cb[-1] + 4608)
    cb.append(ndp)
    return cb


def build_nc(gsizes, debug_taps=False):
    """gsizes: tuple of per-doc group slot counts, gsizes[0] == 192, rest even."""
    assert gsizes[0] == G0 and all(g % 2 == 0 for g in gsizes[1:])
    ngrp = len(gsizes)
    ndp = BD * sum(gsizes)              # doc-position columns per block
    reg_off = np.cumsum([0] + [BD * g for g in gsizes]).tolist()
    cb = _chunks(ndp)

    nc = bacc.Bacc(
        "TRN2",
        target_bir_lowering=False,
        debug=False,
        num_devices=NCORES,
    )

    # qlhsT[b]: [KP, 2, ROWS] fp8 pair-major weights per block
    qlhsT_d = nc.dram_tensor("qlhsT", [NBLK, KP, 2 * ROWS], FP8, kind="ExternalInput")
    # rhs chunk-major: chunk c holds [KP, w_c * 2] fp8 (pair slot innermost)
    rhs_ds = [
        nc.dram_tensor(f"rhs{i}", [KP, (cb[i + 1] - cb[i]) * 2], FP8,
                       kind="ExternalInput")
        for i in range(len(cb) - 1)
    ]
    sel_d = nc.dram_tensor("sel", [NBLK, ROWS, QPC], BF16, kind="ExternalInput")
    qclsT_d = nc.dram_tensor("qclsT", [CLS_D // 128, 128, QPC], BF16, kind="ExternalInput")
    dclsT_d = nc.dram_tensor("dclsT", [CLS_D // 128, 128, BD], BF16, kind="ExternalInput")
    idp_d = nc.dram_tensor("idp", [128, 2 * 128], FP8, kind="ExternalInput")
    out_d = nc.dram_tensor("out", [QPC, BD], F32, kind="ExternalOutput")
    if debug_taps:
        r0dbg_d = nc.dram_tensor("r0dbg", [128, NBLK * BD * gsizes[0]], FP8, kind="ExternalOutput")
        tokdbg_d = nc.dram_tensor("tokdbg", [128, NBLK * BD], BF16, kind="ExternalOutput")
        tdbg_d = nc.dram_tensor("tdbg", [128, ngrp * NBLK * BD], F32, kind="ExternalOutput")

    with tile.TileContext(nc) as tc, ExitStack() as ctx:
        const = ctx.enter_context(tc.tile_pool(name="const", bufs=1))
        psum = ctx.enter_context(tc.tile_pool(name="psum", bufs=7, space="PSUM"))
        tpsum = ctx.enter_context(tc.tile_pool(name="tpsum", bufs=1, space="PSUM"))
        work = ctx.enter_context(tc.tile_pool(name="work", bufs=1))

        # --- SBUF tiles ---
        rhs_t = const.tile([KP, 2 * ndp], FP8, tag="rhs")       # [p, (col, pair)]
        qlhsT_t = const.tile([KP, NBLK * 2 * ROWS], FP8, tag="qlhsT")
        sel_t = const.tile([ROWS, NBLK * QPC], BF16, tag="sel")
        qclsT_t = const.tile([128, 6 * QPC], BF16, tag="qclsT")
        dclsT_t = const.tile([128, 6 * BD], BF16, tag="dclsT")
        idp_t = const.tile([128, 2 * 128], FP8, tag="idp")
        # relu'd decode, raw PSUM column order:
        #   r0[p, (block, group, doc-in-group, phase)]   (phase innermost)
        #   r1+[p, (block, doc, slot)]                   (slot innermost)
        r_ts = []
        for r in range(ngrp):
            r_t = const.tile(
                [128, NBLK * BD * gsizes[r]], FP8, tag=f"r{r}", name=f"r{r}",
            )
            r_ts.append(r_t)

        # --- input DMA: all rhs on the scalar HWDGE queue (the sync queue
        # trickles); small tensors on gpsimd SWDGE ---
        for b in range(NBLK):
            nc.scalar.dma_start(
                qlhsT_t[:, b * 2 * ROWS:(b + 1) * 2 * ROWS], qlhsT_d[b]
            )
        for i in range(len(cb) - 1):
            nc.scalar.dma_start(rhs_t[:, 2 * cb[i]:2 * cb[i + 1]], rhs_ds[i][:])
        nc.gpsimd.dma_start(idp_t[:], idp_d[:])
        for b in range(NBLK):
            nc.gpsimd.dma_start(sel_t[:, b * QPC:(b + 1) * QPC], sel_d[b])
        for k in range(6):
            nc.gpsimd.dma_start(qclsT_t[:, k * QPC:(k + 1) * QPC], qclsT_d[k])
            nc.gpsimd.dma_start(dclsT_t[:, k * BD:(k + 1) * BD], dclsT_d[k])

        # One PSUM bank: rotating T accumulator [128, 2, 128] fp32 (1 KB) +
        # the [8, 128] output accumulator in the bank's upper half
        t_tile = tpsum.tile([128, NBLK + 1, BD], F32, tag="T")
        out_ps = t_tile[0:QPC, NBLK, :]

        rhs_pairs = rhs_t[:].rearrange("p (n o) -> p o n", o=2)

        def aug_mm(ps_slice, b, c0, cw):
            nc.tensor.matmul(
                ps_slice,
                qlhsT_t[:, b * 2 * ROWS:(b + 1) * 2 * ROWS].rearrange(
                    "p (o m) -> p o m", o=2),
                rhs_pairs[:, :, c0:c0 + cw],
                start=True, stop=True,
                perf_mode=mybir.MatmulPerfMode.DoubleRow,
            )

        # --- aug matmuls + relu decode at single-bank granularity; all of
        # block 0 first, then block 1 (stationary weights change once; DR
        # LDWEIGHTS is not hidden when weights alternate).  T matmuls for
        # block 0 interleave into the block-1 sweep. ---
        dgrp = 3 * 512 // G0                      # 8 docs per 3-bank group
        ng0 = BD // dgrp                          # 16 groups of 3 banks
        nbk0 = 3 * ng0                            # 48 banks per block
        idp_ap = idp_t[:].rearrange("p (o m) -> p o m", o=2)
        t_starts = [False] * ngrp

        def t0_phases(pps, bsel):
            """T0 phase-pair matmuls restricted to block bsel's bg range."""
            npp0 = gsizes[0] // 2
            for pp in pps:
                rhs_ap = r_ts[0][:].rearrange(
                    "p (bg pp dl par) -> p pp par bg dl",
                    pp=npp0, dl=dgrp, par=2,
                )[:, pp, :, bsel * ng0:(bsel + 1) * ng0, :]
                nc.tensor.matmul(
                    t_tile[:, bsel, :], idp_ap, rhs_ap,
                    start=(not t_starts[0] and pp == pps[0]),
                    stop=(bsel == NBLK - 1 and pp == pps[-1]),
                    perf_mode=mybir.MatmulPerfMode.DoubleRow,
                )
            t_starts[0] = True

        ralt = 0
        npp0 = gsizes[0] // 2
        t0_batch = [list(range(i, min(i + 8, npp0))) for i in range(0, npp0, 8)]
        for b in range(NBLK):
            bi = 0
            for g in range(ng0):
                for k in range(3):
                    ps = psum.tile([128, 512], F32, tag="aug")
                    aug_mm(ps[:], b, g * 1536 + k * 512, 512)
                    off = (b * nbk0 + g * 3 + k) * 512
                    dst = r_ts[0][:, off:off + 512]
                    if ralt % 2 == 0:
                        nc.scalar.activation(
                            dst, ps[:], mybir.ActivationFunctionType.Relu)
                    else:
                        nc.vector.tensor_scalar_max(dst, ps[:], 0.0)
                    ralt += 1
                if b == 1 and g % 2 == 1 and bi < len(t0_batch):
                    t0_phases(t0_batch[bi], 0)
                    bi += 1
            if b == 1:
                while bi < len(t0_batch):
                    t0_phases(t0_batch[bi], 0)
                    bi += 1
            for r in range(1, ngrp):
                gr = gsizes[r]
                for c0 in range(0, BD * gr, 512):
                    cw = min(512, BD * gr - c0)
                    ps = psum.tile([128, 512], F32, tag="aug")
                    aug_mm(ps[:, 0:cw], b, reg_off[r] + c0, cw)
                    dst = r_ts[r][:, b * BD * gr + c0:b * BD * gr + c0 + cw]
                    if ralt % 2 == 0:
                        nc.scalar.activation(
                            dst, ps[:, 0:cw], mybir.ActivationFunctionType.Relu)
                    else:
                        nc.vector.tensor_scalar_max(dst, ps[:, 0:cw], 0.0)
                    ralt += 1

        # T0 for block 1, then regions 1+ for both blocks; T accumulator bank
        # is rotated through SBUF copies (tsb) between groups
        t0_phases(list(range(npp0)), 1)
        tsb = []
        cp = work.tile([128, ngrp, NBLK * BD], BF16, tag="tsb_cp")
        nc.vector.tensor_copy(cp[:, 0, :], t_tile[:, 0:NBLK, :])
        tsb.append(cp[:, 0, :])
        for r in range(1, ngrp):
            gr = gsizes[r]
            npp = gr // 2
            for pp in range(npp):
                rhs_ap = r_ts[r][:].rearrange(
                    "p (b pp d par) -> p pp par b d",
                    b=NBLK, d=BD, par=2,
                )[:, pp, :, :, :]
                nc.tensor.matmul(
                    t_tile[:, 0:NBLK, :], idp_ap, rhs_ap,
                    start=(pp == 0), stop=(pp == npp - 1),
                    perf_mode=mybir.MatmulPerfMode.DoubleRow,
                )
            if r % 2:
                nc.scalar.copy(cp[:, r, :], t_tile[:, 0:NBLK, :])
            else:
                nc.vector.tensor_copy(cp[:, r, :], t_tile[:, 0:NBLK, :])
            tsb.append(cp[:, r, :])

        # --- group max (exact reproduction of the reference per-row max) ---
        tok_t = work.tile([128, NBLK * BD], BF16, tag="tok")
        if ngrp == 1:
            nc.vector.tensor_copy(tok_t[:], tsb[0])
        else:
            acc = tsb[0]
            for r in range(1, ngrp):
                nxt = tok_t if r == ngrp - 1 else work.tile(
                    [128, NBLK * BD], BF16, tag=f"tmax{r}", name=f"tmax{r}"
                )
                nc.vector.tensor_tensor(
                    nxt[:], tsb[r], acc, op=mybir.AluOpType.max
                )
                acc = nxt[:]
        # CLS matmuls: must start the out_ps accumulation group AFTER every
        # other start= in its PSUM bank (start clears has_written bank-wide)
        for k in range(6):
            nc.tensor.matmul(
                out_ps[:],
                qclsT_t[:, k * QPC:(k + 1) * QPC],
                dclsT_t[:, k * BD:(k + 1) * BD],
                start=(k == 0),
                stop=False,
            )
        # --- weighted token sum into out_ps ---
        for b in range(NBLK):
            nc.tensor.matmul(
                out_ps[:],
                sel_t[:, b * QPC:(b + 1) * QPC],
                tok_t[:, b * BD:(b + 1) * BD],
                start=False,
                stop=(b == NBLK - 1),
            )

        outsb = work.tile([QPC, BD], F32, tag="outsb")
        nc.scalar.copy(outsb[:], out_ps[:])
        nc.sync.dma_start(out_d[:], outsb[:])
        if debug_taps:
            nc.sync.dma_start(r0dbg_d[:], r_ts[0][:])
            nc.sync.dma_start(tokdbg_d[:], tok_t[:])
            tsbf = work.tile([128, ngrp * NBLK * BD], F32, tag="tsbf")
            for r in range(ngrp):
                nc.vector.tensor_copy(
                    tsbf[:, r * NBLK * BD:(r + 1) * NBLK * BD], cp[:, r, :])
            nc.sync.dma_start(tdbg_d[:], tsbf[:])

    nc.compile()
    return nc


_NC_CACHE = {}


def _get_nc(gsizes, debug_taps=False):
    key = (gsizes, debug_taps)
    if key not in _NC_CACHE:
        _NC_CACHE[key] = build_nc(gsizes, debug_taps)
    return _NC_CACHE[key]


def _digit_onehot(ids, scale):
    ids = ids.astype(np.int64)
    oh = np.zeros(ids.shape + (KD,), np.float32)
    flat = oh.reshape(-1, KD)
    fid = ids.reshape(-1)
    idx = np.arange(fid.size)
    for t in range(NDIG):
        flat[idx, t * DIG + (fid // (DIG ** t)) % DIG] = scale
    return oh


def _doc_groups(did):
    """Duplicate-rank grouping of doc positions (doc-side data only)."""
    ranks = np.zeros_like(did, dtype=np.int64)
    for d in range(BD):
        seen = {}
        for j in range(LD):
            v = int(did[d, j])
            r = seen.get(v, 0)
            seen[v] = r + 1
            ranks[d, j] = r
    nrank = int(ranks.max()) + 1
    gsizes = []
    pos = []
    for r in range(nrank):
        cnt = (ranks == r).sum(axis=1)
        gr = G0 if r == 0 else max(2, int(np.ceil(cnt.max() / 2)) * 2)
        p = np.full((BD, gr), -1, np.int64)
        for d in range(BD):
            js = np.nonzero(ranks[d] == r)[0]
            p[d, :len(js)] = js
        gsizes.append(gr)
        pos.append(p)
    return tuple(gsizes), pos


def _hilo8(x):
    hi = x.astype(E4)
    lo = (x - hi.astype(np.float32)).astype(E4)
    return hi.astype(np.float32), lo.astype(np.float32)


def make_in_maps(qte, dte, qce, dce, qid, did, qam):
    # SEP mask + CLS drop -> per-token weights
    sep = qam.sum(1) - 1
    qm = qam.astype(np.float32).copy()
    qm[np.arange(BQ), sep] = 0.0
    w = qm.copy()
    w[:, 0] = 0.0

    gsizes, pos = _doc_groups(did)
    ndp = BD * sum(gsizes)
    cb = _chunks(ndp)

    # doc-side feature pairs [KP, ndp, 2] fp8: (d8, d8) / onehot pairs / (1, 1)
    doh = _digit_onehot(did, 1.0)                  # [BD, LD, KD]
    d8 = dte.astype(E4).astype(np.float32)         # [BD, LD, TOK_D]
    dfeat = np.zeros((KP, BD * LD, 2), np.float32)
    d8f = d8.transpose(2, 0, 1).reshape(TOK_D, BD * LD)
    dfeat[0:TOK_D, :, 0] = d8f
    dfeat[0:TOK_D, :, 1] = d8f
    dohf = doh.transpose(2, 0, 1).reshape(KD, BD * LD)
    for j in range(KD // 2):
        dfeat[TOK_D + j, :, 0] = dohf[2 * j]
        dfeat[TOK_D + j, :, 1] = dohf[2 * j + 1]
    dfeat[KP - 1, :, :] = 2.0

    rhs = np.zeros((KP, ndp, 2), E4)
    off = 0
    dgrp0 = 8
    for r, gr in enumerate(gsizes):
        idx = pos[r]                               # [BD, gr], -1 pad
        if r == 0:
            # column order (g, pp, dl, par): doc = 8g+dl, slot = 2pp+par
            iv = idx.reshape(BD // dgrp0, dgrp0, gr // 2, 2)   # [g, dl, pp, par]
            iv = iv.transpose(0, 2, 1, 3)                      # [g, pp, dl, par]
        else:
            # column order (pp, d, par)
            iv = idx.reshape(BD, gr // 2, 2)                   # [d, pp, par]
            iv = iv.transpose(1, 0, 2)                         # [pp, d, par]
        iv = np.ascontiguousarray(iv)
        docof = {0: (np.arange(BD).reshape(BD // dgrp0, dgrp0, 1, 1)
                     .transpose(0, 2, 1, 3) if False else None)}
        # doc index per column, matching iv's layout
        if r == 0:
            dv = np.broadcast_to(
                np.arange(BD).reshape(BD // dgrp0, 1, dgrp0, 1),
                iv.shape)
        else:
            dv = np.broadcast_to(np.arange(BD).reshape(1, BD, 1), iv.shape)
        flat_i = iv.reshape(-1)
        flat_d = dv.reshape(-1)
        src = np.where(flat_i >= 0, flat_d * LD + np.maximum(flat_i, 0), 0)
        block = dfeat[:, src, :].astype(E4)
        block[:, flat_i < 0, :] = 0
        rhs[:, off:off + BD * gr] = block
        off += BD * gr

    qoh = _digit_onehot(qid, C)                    # [BQ, LQ, KD]
    dclsT = np.ascontiguousarray(
        dce.T.reshape(CLS_D // 128, 128, BD)).astype(ml_dtypes.bfloat16)
    idp = np.zeros((128, 2 * 128), dtype=E4)
    for p in range(128):
        idp[p, p] = 1.0
        idp[p, 128 + p] = 1.0

    rhs_chunks = {
        f"rhs{i}": np.ascontiguousarray(
            rhs[:, cb[i]:cb[i + 1], :].reshape(KP, -1))
        for i in range(len(cb) - 1)
    }

    in_maps = []
    for c in range(NCORES):
        qs = slice(c * QPC, (c + 1) * QPC)
        qte_c, qoh_c, w_c = qte[qs], qoh[qs], w[qs]

        qlhsT = np.zeros((NBLK, KP, 2, ROWS), np.float32)
        for b in range(NBLK):
            blk = qte_c[b * 4:(b + 1) * 4].reshape(ROWS, TOK_D)
            qh, ql = _hilo8(blk)
            qlhsT[b, 0:TOK_D, 0] = qh.T
            qlhsT[b, 0:TOK_D, 1] = ql.T
            ohb = qoh_c[b * 4:(b + 1) * 4].reshape(ROWS, KD).T
            for j in range(KD // 2):
                qlhsT[b, TOK_D + j, 0] = ohb[2 * j]
                qlhsT[b, TOK_D + j, 1] = ohb[2 * j + 1]
            qlhsT[b, KP - 1, :, :] = BIAS

        sel = np.zeros((NBLK, ROWS, QPC), np.float32)
        for b in range(NBLK):
            for qq in range(4):
                ql_ = b * 4 + qq
                sel[b, qq * 32:(qq + 1) * 32, ql_] = w_c[ql_]

        qclsT = np.ascontiguousarray(
            qce[qs].T.reshape(CLS_D // 128, 128, QPC)).astype(ml_dtypes.bfloat16)

        im = {
            "qlhsT": qlhsT.reshape(NBLK, KP, 2 * ROWS).astype(E4),
            "sel": sel.astype(ml_dtypes.bfloat16),
            "qclsT": qclsT,
            "dclsT": dclsT,
            "idp": idp,
        }
        im.update(rhs_chunks)
        in_maps.append(im)
    return gsizes, in_maps


def run(gsizes, in_maps, trace=False, debug_taps=False, **kwargs):
    nc = _get_nc(gsizes, debug_taps)
    return run_bass_kernel_spmd(
        nc, in_maps, core_ids=list(range(NCORES)), trace=trace, **kwargs
    )


def kernel(
    query_tok_embs,
    doc_tok_embs,
    query_cls_emb,
    doc_cls_emb,
    query_input_ids,
    doc_input_ids,
    query_attention_mask,
):
    qte = np.ascontiguousarray(np.asarray(query_tok_embs, np.float32))
    dte = np.ascontiguousarray(np.asarray(doc_tok_embs, np.float32))
    qce = np.ascontiguousarray(np.asarray(query_cls_emb, np.float32))
    dce = np.ascontiguousarray(np.asarray(doc_cls_emb, np.float32))
    qid = np.asarray(query_input_ids).astype(np.int64)
    did = np.asarray(doc_input_ids).astype(np.int64)
    qam = np.asarray(query_attention_mask).astype(np.int64)

    gsizes, in_maps = make_in_maps(qte, dte, qce, dce, qid, did, qam)
    res = run(gsizes, in_maps)
    out = np.concatenate([r["out"] for r in res.results], axis=0)
    return np.ascontiguousarray(out.astype(np.float32))
